# revision 9
# baseline (speedup 1.0000x reference)
"""Multi-head causal attention (B=1, S=4096, D=1024, H=16, HD=64) on 8
Trainium2 NeuronCores.

Sharding: head-parallel - 16 heads / 8 cores = 2 heads per core (one
128-channel slice of the QKV/output projections per core).

v2 design notes (vs the f32r baseline at ~310us):
  * All matmul operands are fp16 (psum stays f32). fp16 keeps 1 cyc/row
    streaming, halves LDWEIGHTS (FWL), halves SBUF/DMA bytes, and avoids
    the f32r 4-cyc/row penalty on narrow diagonal tiles. Accuracy budget
    (rel tol 2e-2) has ~20x margin at fp16.
  * Scores use FULL-ARRAY (128-row) stationary kT tiles: trace analysis
    showed full-row LDWEIGHTS pulls ahead under in-flight matmuls
    (phase-1 chains ran at 227ns/MM = stream rate) while the baseline's
    64-row strip LDWs cannot and serialize (~600ns/pair). The two heads
    are separated by ZERO-PADDING the moving Q operand instead: qpad
    holds [qA;0] and [0;qB] blocks so one kT_j stationary serves both
    heads with plain full-array matmuls.
  * Causal masking is additive (-1e5) on the PSUM scores via DVE before
    the exp, removing the GpSimd mask-multiply from the pt->PV critical
    path.
  * Softmax denominator: v_aug = [V_A | ones | V_B] rider (as baseline);
    normalization is DVE reciprocal + multiply (no Ln/Exp table games),
    with two ScalarE Copy ops for the cross-partition l moves.
  * Output projection is interleaved per query-block into the next
    block's attention, evicted on GpSimd, written as fp16 partials.
  * Emission software-pipelines scores(j+1) ahead of PV(j) so the PE
    queue always has independent work while ScalarE runs exp (ScalarE is
    within ~5% of TensorE here; exp is ~110us/core of irreducible work).
"""

import os
import sys

import numpy as np

for _p in ("/opt/trn_rl_repo", "/root/.axon_site/_ro/trn_rl_repo"):
    if os.path.isdir(_p) and _p not in sys.path:
        sys.path.insert(0, _p)

from contextlib import ExitStack

import concourse.bass as bass
import concourse.tile as tile
from concourse import bacc, bass_utils, mybir
from concourse.masks import make_identity

# Problem shape (hardcoded per the harness contract).
B, S, D, H = 1, 4096, 1024, 16
HD = D // H          # 64
NCORES = 8
HPC = H // NCORES    # 2 heads per core
M = HPC * HD         # 128 channels per core
SBK = 512            # query/sequence block size
NSB = S // SBK       # 8
DBK = 128            # d block size
NDB = D // DBK       # 8
JBK = 128            # key block size
NJT = S // JBK       # 32 j-tiles
VW = 3 * HD          # v_aug row width per j-tile: [V_A | ones | V_B]
NEG = -1.0e5         # additive causal mask value (pre-softmax)

F32 = mybir.dt.float32
F16 = mybir.dt.float16

_CACHE = {}


def _build_nc():
    nc = bacc.Bacc("TRN2", target_bir_lowering=False, debug=False,
                   num_devices=NCORES)

    xT = nc.dram_tensor("xT", [D, S], F16, kind="ExternalInput").ap()
    wq = nc.dram_tensor("wq", [D, M], F16, kind="ExternalInput").ap()
    wk = nc.dram_tensor("wk", [D, M], F16, kind="ExternalInput").ap()
    wv = nc.dram_tensor("wv", [D, M], F16, kind="ExternalInput").ap()
    wo = nc.dram_tensor("wo", [M, D], F16, kind="ExternalInput").ap()
    ones = nc.dram_tensor("ones", [128, HD], F16, kind="ExternalInput").ap()
    nmask = nc.dram_tensor("nmask", [JBK, 2 * JBK], F32,
                           kind="ExternalInput").ap()
    outp = nc.dram_tensor("outp", [D, S], F16, kind="ExternalOutput").ap()

    with tile.TileContext(nc) as tc:
        with ExitStack() as ctx:
            _emit(ctx, tc, nc, xT, wq, wk, wv, wo, ones, nmask, outp)
    nc.compile()
    return nc


def _emit(ctx, tc, nc, xT, wq, wk, wv, wo, ones, nmask, outp):
    const = ctx.enter_context(tc.tile_pool(name="const", bufs=1))
    persist = ctx.enter_context(tc.tile_pool(name="persist", bufs=1))
    xt_pool = ctx.enter_context(tc.tile_pool(name="xt", bufs=16))
    vt_pool = ctx.enter_context(tc.tile_pool(name="vt", bufs=2))
    pt_pool = ctx.enter_context(tc.tile_pool(name="pt", bufs=4))
    out_pool = ctx.enter_context(tc.tile_pool(name="outt", bufs=4))
    small = ctx.enter_context(tc.tile_pool(name="small", bufs=2))
    # PSUM budget (16KB/partition = 8 banks):
    #   psA tag "sc":  [128,1024] f32 = 4KB x2 bufs = 8KB (scores; phase1 q/k)
    #   psB tag "acc": [128,512]  f32 = 2KB x2 bufs = 4KB (acc A/B; phase1 v)
    #   psB tag "po":  [128,512]  f32 = 2KB x2 bufs = 4KB (proj out)
    psA = ctx.enter_context(tc.tile_pool(name="psA", bufs=2, space="PSUM"))
    psB = ctx.enter_context(tc.tile_pool(name="psB", bufs=2, space="PSUM"))

    # ---- constants / persistent SBUF ----
    ident = const.tile([128, 128], F16)
    make_identity(nc, ident)

    wq_sb = const.tile([128, D], F16)    # 8 d-tiles side by side [d, m]
    wk_sb = const.tile([128, D], F16)
    wv_sb = const.tile([128, D], F16)
    wo_sb = const.tile([128, D], F16)    # [m, o]
    mask_sb = const.tile([JBK, 2 * JBK], F32)

    def _w_chunk(w_sb, w_dram, c, n=2):
        w_r = w_dram.rearrange("(d p) m -> p d m", p=DBK)
        w_sb_r = w_sb[:].rearrange("p (d m) -> p d m", d=NDB)
        nc.sync.dma_start(out=w_sb_r[:, c:c + n, :], in_=w_r[:, c:c + n, :])

    for c in (0, 2, 4, 6):
        for w_sb, w_dram in ((wq_sb, wq), (wk_sb, wk), (wv_sb, wv)):
            _w_chunk(w_sb, w_dram, c, 2)

    def _late_consts():
        for c in range(4):
            nc.sync.dma_start(out=wo_sb[:, bass.ts(c, 256)],
                              in_=wo[:, bass.ts(c, 256)])
        nc.sync.dma_start(out=mask_sb[:], in_=nmask[:])

    # qpad: per query-block layout [qA(512) ; zeros] | [zeros ; qB(512)]
    qpad = persist.tile([128, 2 * S], F16)
    kT_sb = persist.tile([128, S], F16)
    v_aug = persist.tile([128, NJT * VW], F16)
    attnT = persist.tile([128, S], F16)

    nc.gpsimd.memset(qpad[:], 0.0)

    # v_aug per j-tile: [V_A | ones | V_B]; head A reads cols 0:128
    # (denominator in acc rows 64:128), head B reads cols 64:192
    # (denominator in acc rows 0:64). ones filled by one broadcast DMA.
    v_aug_r = v_aug[:].rearrange("p (t c w) -> p t c w", c=3, w=HD)
    ones_bcast = bass.AP(
        tensor=ones.tensor, offset=0,
        ap=[[HD, 128], [0, NJT], [1, HD]])
    nc.sync.dma_start(out=v_aug_r[:, :, 1, :], in_=ones_bcast)

    def phase1(sb):
        """QKV projections for s-block sb (512 sequence positions)."""
        q_ps = psA.tile([128, SBK], F32, tag="sc")
        k_ps = psA.tile([128, SBK], F32, tag="sc")
        v_ps = psB.tile([128, SBK], F32, tag="acc")
        for d in range(NDB):
            xt = xt_pool.tile([128, SBK], F16, tag="xt")
            nc.gpsimd.dma_start(out=xt[:],
                                in_=xT[bass.ts(d, DBK), bass.ts(sb, SBK)])
            st, sp = d == 0, d == NDB - 1
            nc.tensor.matmul(q_ps[:], lhsT=wq_sb[:, bass.ts(d, M)],
                             rhs=xt[:], start=st, stop=sp)
            nc.tensor.matmul(k_ps[:], lhsT=wk_sb[:, bass.ts(d, M)],
                             rhs=xt[:], start=st, stop=sp)
            nc.tensor.matmul(v_ps[:], lhsT=wv_sb[:, bass.ts(d, M)],
                             rhs=xt[:], start=st, stop=sp)
        # evictions: qA/qB into zero-padded halves + k on ScalarE (idle
        # during phase 1), v on DVE, then PE-transpose into v_aug slots.
        nc.scalar.copy(qpad[0:64, 2 * sb * SBK:(2 * sb + 1) * SBK],
                       q_ps[0:64, :])
        nc.scalar.copy(qpad[64:128, (2 * sb + 1) * SBK:(2 * sb + 2) * SBK],
                       q_ps[64:128, :])
        nc.scalar.copy(kT_sb[:, bass.ts(sb, SBK)], k_ps[:])
        vt = vt_pool.tile([128, SBK], F16)
        nc.vector.tensor_copy(vt[:], v_ps[:])
        for t in range(SBK // JBK):
            jt = sb * (SBK // JBK) + t
            tp = psB.tile([128, JBK], F16, tag="po")
            nc.tensor.transpose(tp[:], vt[:, bass.ts(t, JBK)], ident[:])
            nc.vector.tensor_copy(v_aug_r[:, jt, 0::2, :], tp[:])

    def emit_proj(qb):
        """Output-projection partial for query block qb -> DRAM (fp16)."""
        qsl = bass.ts(qb, SBK)
        for ob in range(NDB):
            po = psB.tile([128, SBK], F32, tag="po")
            nc.tensor.matmul(po[:], lhsT=wo_sb[:, bass.ts(ob, 128)],
                             rhs=attnT[:, qsl], start=True, stop=True)
            ot = out_pool.tile([128, SBK], F16)
            nc.vector.tensor_copy(ot[:], po[:])
            nc.sync.dma_start(out=outp[bass.ts(ob, 128), qsl], in_=ot[:])

    def attention(qb, pending_proj=None):
        """Causal attention for query block qb (both heads)."""
        nj = 4 * (qb + 1)
        acc_A = psB.tile([128, SBK], F32, tag="acc")
        acc_B = psB.tile([128, SBK], F32, tag="acc")

        def emit_scores(j):
            r = j - (nj - 4)
            off = 128 * r if r > 0 else 0
            sc = psA.tile([128, 2 * SBK], F32, tag="sc")
            kslice = kT_sb[:, bass.ts(j, JBK)]
            qa = qpad[:, 2 * qb * SBK + off:(2 * qb + 1) * SBK]
            qb_ap = qpad[:, (2 * qb + 1) * SBK + off:(2 * qb + 2) * SBK]
            nc.tensor.matmul(sc[:, off:SBK], lhsT=kslice, rhs=qa,
                             start=True, stop=True)
            nc.tensor.matmul(sc[:, SBK + off:2 * SBK], lhsT=kslice,
                             rhs=qb_ap, start=True, stop=True)
            if r >= 0:
                # additive causal mask on the [128,128] diagonal strip of
                # both heads (one 3D DVE op, pre-exp, on PSUM).
                dlo = 128 * r
                sc3 = bass.AP(tensor=sc.tensor, offset=sc.offset + dlo,
                              ap=[list(sc.ap[0]), [SBK, 2], [1, JBK]])
                m3 = mask_sb[:].rearrange("p (b c) -> p b c", b=2)
                nc.vector.tensor_add(sc3, sc3, m3)
            return sc, off

        cur = emit_scores(0)
        for j in range(nj):
            nxt = emit_scores(j + 1) if j + 1 < nj else None
            if pending_proj is not None and j == 2:
                emit_proj(pending_proj)
                pending_proj = None
            sc, off = cur
            pt = pt_pool.tile([128, 2 * SBK], F16, tag="pt")
            scale = float(1.0 / np.sqrt(HD))
            if off == 0:
                nc.scalar.activation(pt[:], sc[:],
                                     mybir.ActivationFunctionType.Exp,
                                     scale=scale)
            else:
                w = SBK - off
                sc2 = bass.AP(tensor=sc.tensor, offset=sc.offset + off,
                              ap=[list(sc.ap[0]), [SBK, 2], [1, w]])
                pt2 = bass.AP(tensor=pt.tensor, offset=pt.offset + off,
                              ap=[list(pt.ap[0]), [SBK, 2], [1, w]])
                nc.scalar.activation(pt2, sc2,
                                     mybir.ActivationFunctionType.Exp,
                                     scale=scale)
            st, sp = j == 0, j == nj - 1
            vb = j * VW
            nc.tensor.matmul(acc_A[:, off:SBK],
                             lhsT=v_aug[:, vb:vb + 128],
                             rhs=pt[:, off:SBK], start=st, stop=sp)
            nc.tensor.matmul(acc_B[:, off:SBK],
                             lhsT=v_aug[:, vb + HD:vb + VW],
                             rhs=pt[:, SBK + off:2 * SBK],
                             start=st, stop=sp)
            cur = nxt
        if pending_proj is not None:
            emit_proj(pending_proj)
        return acc_A, acc_B

    def normalize(qb, acc_A, acc_B):
        """attnT = acc_out / l. Head A: out rows 0:64, l rows 64:128;
        head B flipped. Cross-partition l moves on ScalarE Copy, then one
        DVE reciprocal + two DVE multiplies."""
        qsl = bass.ts(qb, SBK)
        lt = small.tile([128, SBK], F32, tag="lt")
        nc.scalar.copy(lt[0:64, :], acc_A[HD:2 * HD, :])
        nc.scalar.copy(lt[64:128, :], acc_B[0:HD, :])
        li = small.tile([128, SBK], F32, tag="li")
        nc.vector.reciprocal_approx_fast(out=li[:], in_=lt[:])
        nc.vector.tensor_mul(attnT[0:64, qsl], acc_A[0:HD, :], li[0:64, :])
        nc.vector.tensor_mul(attnT[64:128, qsl], acc_B[HD:2 * HD, :],
                             li[64:128, :])

    # ---- emission ----
    for sb in range(NSB):
        phase1(sb)
        if sb == 0:
            _late_consts()
    for qb in range(NSB):
        accs = attention(qb, pending_proj=qb - 1 if qb > 0 else None)
        normalize(qb, *accs)
    emit_proj(NSB - 1)


def _host_prep(x, Wq, Wk, Wv, Wo):
    xT = np.ascontiguousarray(x.reshape(S, D).T).astype(np.float16)
    jj = np.arange(JBK)[:, None]
    qq = np.arange(JBK)[None, :]
    tri = np.where(jj <= qq, np.float32(0.0), np.float32(NEG))
    nmask = np.concatenate([tri, tri], axis=1)
    in_maps = []
    for c in range(NCORES):
        sl = slice(c * M, (c + 1) * M)
        in_maps.append({
            "xT": xT,
            "wq": np.ascontiguousarray(Wq[sl, :].T).astype(np.float16),
            "wk": np.ascontiguousarray(Wk[sl, :].T).astype(np.float16),
            "wv": np.ascontiguousarray(Wv[sl, :].T).astype(np.float16),
            "wo": np.ascontiguousarray(Wo[:, sl].T).astype(np.float16),
            "ones": np.ones((128, HD), dtype=np.float16),
            "nmask": np.ascontiguousarray(nmask),
        })
    return in_maps


def _run(inputs, trace=False):
    x = np.asarray(inputs["x"], dtype=np.float32)
    Wq = np.asarray(inputs["Wq"], dtype=np.float32)
    Wk = np.asarray(inputs["Wk"], dtype=np.float32)
    Wv = np.asarray(inputs["Wv"], dtype=np.float32)
    Wo = np.asarray(inputs["Wo"], dtype=np.float32)

    if "nc" not in _CACHE:
        _CACHE["nc"] = _build_nc()
    nc = _CACHE["nc"]

    in_maps = _host_prep(x, Wq, Wk, Wv, Wo)
    res = bass_utils.run_bass_kernel_spmd(
        nc, in_maps, core_ids=list(range(NCORES)), trace=trace)
    partial = np.zeros((D, S), dtype=np.float32)
    for c in range(NCORES):
        partial += res.results[c]["outp"].astype(np.float32)
    out = partial.T.astype(np.float32).reshape(B, S, D)
    return out, res


def kernel(x, mask, Wq, Wk, Wv, Wo):
    mask = np.asarray(mask)
    causal = np.tril(np.ones((S, S), dtype=bool))
    if mask.reshape(S, S).shape == causal.shape and bool(
            np.array_equal(mask.reshape(S, S), causal)):
        out, _ = _run({"x": x, "Wq": Wq, "Wk": Wk, "Wv": Wv, "Wo": Wo})
        return out
    # safety net for a non-causal mask: exact numpy fallback
    return _numpy_ref(np.asarray(x, np.float32), mask,
                      np.asarray(Wq, np.float32), np.asarray(Wk, np.float32),
                      np.asarray(Wv, np.float32), np.asarray(Wo, np.float32))


def _numpy_ref(x, mask, Wq, Wk, Wv, Wo):
    xf = x.reshape(S, D)
    q = xf @ Wq.T
    k = xf @ Wk.T
    v = xf @ Wv.T
    m2 = mask.reshape(S, S)
    o = np.empty((S, D), dtype=np.float32)
    for h in range(H):
        hs = slice(h * HD, (h + 1) * HD)
        sc = (q[:, hs] @ k[:, hs].T) / np.sqrt(np.float32(HD))
        sc = np.where(m2, sc, np.float32(-1e9))
        sc -= sc.max(axis=-1, keepdims=True)
        p = np.exp(sc)
        p /= p.sum(axis=-1, keepdims=True)
        o[:, hs] = p @ v[:, hs]
    return (o @ Wo.T).astype(np.float32).reshape(B, S, D)


# revision 12
# speedup vs baseline: 1.1222x; 1.1222x over previous
"""Multi-head causal attention (B=1, S=4096, D=1024, H=16, HD=64) on 8
Trainium2 NeuronCores.

Sharding: head-parallel - 16 heads / 8 cores = 2 heads per core (one
128-channel slice of the QKV/output projections per core).

v2 design notes (vs the f32r baseline at ~310us):
  * All matmul operands are fp16 (psum stays f32). fp16 keeps 1 cyc/row
    streaming, halves LDWEIGHTS (FWL), halves SBUF/DMA bytes, and avoids
    the f32r 4-cyc/row penalty on narrow diagonal tiles. Accuracy budget
    (rel tol 2e-2) has ~20x margin at fp16.
  * Scores use FULL-ARRAY (128-row) stationary kT tiles: trace analysis
    showed full-row LDWEIGHTS pulls ahead under in-flight matmuls
    (phase-1 chains ran at 227ns/MM = stream rate) while the baseline's
    64-row strip LDWs cannot and serialize (~600ns/pair). The two heads
    are separated by ZERO-PADDING the moving Q operand instead: qpad
    holds [qA;0] and [0;qB] blocks so one kT_j stationary serves both
    heads with plain full-array matmuls.
  * Causal masking is additive (-1e5) on the PSUM scores via DVE before
    the exp, removing the GpSimd mask-multiply from the pt->PV critical
    path.
  * Softmax denominator: v_aug = [V_A | ones | V_B] rider (as baseline);
    normalization is DVE reciprocal + multiply (no Ln/Exp table games),
    with two ScalarE Copy ops for the cross-partition l moves.
  * Output projection is interleaved per query-block into the next
    block's attention, evicted on GpSimd, written as fp16 partials.
  * Emission software-pipelines scores(j+1) ahead of PV(j) so the PE
    queue always has independent work while ScalarE runs exp (ScalarE is
    within ~5% of TensorE here; exp is ~110us/core of irreducible work).
"""

import os
import sys

import numpy as np

for _p in ("/opt/trn_rl_repo", "/root/.axon_site/_ro/trn_rl_repo"):
    if os.path.isdir(_p) and _p not in sys.path:
        sys.path.insert(0, _p)

from contextlib import ExitStack

import concourse.bass as bass
import concourse.tile as tile
from concourse import bacc, bass_utils, mybir
from concourse.masks import make_identity

# Problem shape (hardcoded per the harness contract).
B, S, D, H = 1, 4096, 1024, 16
HD = D // H          # 64
NCORES = 8
HPC = H // NCORES    # 2 heads per core
M = HPC * HD         # 128 channels per core
SBK = 512            # query/sequence block size
NSB = S // SBK       # 8
DBK = 128            # d block size
NDB = D // DBK       # 8
JBK = 128            # key block size
NJT = S // JBK       # 32 j-tiles
VW = 3 * HD          # v_aug row width per j-tile: [V_A | ones | V_B]
NEG = -1.0e5         # additive causal mask value (pre-softmax)

F32 = mybir.dt.float32
F16 = mybir.dt.float16
F32R = mybir.dt.float32r

_CACHE = {}


def _build_nc():
    nc = bacc.Bacc("TRN2", target_bir_lowering=False, debug=False,
                   num_devices=NCORES)

    xT = nc.dram_tensor("xT", [D, S], F16, kind="ExternalInput").ap()
    wq = nc.dram_tensor("wq", [D, M], F16, kind="ExternalInput").ap()
    wk = nc.dram_tensor("wk", [D, M], F16, kind="ExternalInput").ap()
    wv = nc.dram_tensor("wv", [D, M], F16, kind="ExternalInput").ap()
    wo = nc.dram_tensor("wo", [M, D], F16, kind="ExternalInput").ap()
    ones = nc.dram_tensor("ones", [128, HD], F32R, kind="ExternalInput").ap()
    nmask = nc.dram_tensor("nmask", [JBK, 2 * JBK], F32,
                           kind="ExternalInput").ap()
    outp = nc.dram_tensor("outp", [D, S], F16, kind="ExternalOutput").ap()

    with tile.TileContext(nc) as tc:
        with ExitStack() as ctx:
            _emit(ctx, tc, nc, xT, wq, wk, wv, wo, ones, nmask, outp)
    nc.compile()
    return nc


def _emit(ctx, tc, nc, xT, wq, wk, wv, wo, ones, nmask, outp):
    const = ctx.enter_context(tc.tile_pool(name="const", bufs=1))
    persist = ctx.enter_context(tc.tile_pool(name="persist", bufs=1))
    xt_pool = ctx.enter_context(tc.tile_pool(name="xt", bufs=3))
    vt_pool = ctx.enter_context(tc.tile_pool(name="vt", bufs=2))
    pt_pool = ctx.enter_context(tc.tile_pool(name="pt", bufs=4))
    out_pool = ctx.enter_context(tc.tile_pool(name="outt", bufs=4))
    small = ctx.enter_context(tc.tile_pool(name="small", bufs=2))
    # PSUM budget (16KB/partition = 8 banks):
    #   psA tag "sc":  [128,1024] f32 = 4KB x2 bufs = 8KB (scores; phase1 q/k)
    #   psB tag "acc": [128,512]  f32 = 2KB x2 bufs = 4KB (acc A/B; phase1 v)
    #   psB tag "po":  [128,512]  f32 = 2KB x2 bufs = 4KB (proj out)
    psA = ctx.enter_context(tc.tile_pool(name="psA", bufs=2, space="PSUM"))
    psB = ctx.enter_context(tc.tile_pool(name="psB", bufs=2, space="PSUM"))

    # ---- constants / persistent SBUF ----
    ident = const.tile([128, 128], F16)
    make_identity(nc, ident)

    wq_sb = const.tile([128, D], F16)    # 8 d-tiles side by side [d, m]
    wk_sb = const.tile([128, D], F16)
    wv_sb = const.tile([128, D], F16)
    wo_sb = const.tile([128, D], F16)    # [m, o]
    mask_sb = const.tile([JBK, 2 * JBK], F32)

    for w_sb, w_dram in ((wq_sb, wq), (wk_sb, wk), (wv_sb, wv)):
        w_r = w_dram.rearrange("(d p) m -> p d m", p=DBK)
        w_sb_r = w_sb[:].rearrange("p (d m) -> p d m", d=NDB)
        nc.sync.dma_start(out=w_sb_r, in_=w_r)

    def _late_consts():
        for c in range(4):
            nc.sync.dma_start(out=wo_sb[:, bass.ts(c, 256)],
                              in_=wo[:, bass.ts(c, 256)])
        nc.sync.dma_start(out=mask_sb[:], in_=nmask[:])

    # qpad: per query-block layout [qA(512) ; zeros] | [zeros ; qB(512)]
    qpad = persist.tile([128, 2 * S], F16)
    kT_sb = persist.tile([128, S], F16)
    v_aug = persist.tile([128, NJT * VW], F32R)
    attnT = persist.tile([128, S], F16)

    nc.gpsimd.memset(qpad[:], 0.0)

    # v_aug per j-tile: [V_A | ones | V_B]; head A reads cols 0:128
    # (denominator in acc rows 64:128), head B reads cols 64:192
    # (denominator in acc rows 0:64). ones filled by one broadcast DMA.
    v_aug_r = v_aug[:].rearrange("p (t c w) -> p t c w", c=3, w=HD)
    ones_bcast = bass.AP(
        tensor=ones.tensor, offset=0,
        ap=[[HD, 128], [0, NJT], [1, HD]])
    nc.sync.dma_start(out=v_aug_r[:, :, 1, :], in_=ones_bcast)

    def phase1(sb):
        """QKV projections for s-block sb (512 sequence positions)."""
        q_ps = psA.tile([128, SBK], F32, tag="sc")
        k_ps = psA.tile([128, SBK], F32, tag="sc")
        v_ps = psB.tile([128, SBK], F32, tag="acc")
        xt = xt_pool.tile([128, NDB * SBK], F16, tag="xt")
        xt_r = xt[:].rearrange("p (d s) -> p d s", d=NDB)
        xT_r = xT.rearrange("(d p) s -> p d s", p=DBK)
        nc.gpsimd.dma_start(out=xt_r, in_=xT_r[:, :, sb * SBK:(sb + 1) * SBK])
        for d in range(NDB):
            st, sp = d == 0, d == NDB - 1
            nc.tensor.matmul(q_ps[:], lhsT=wq_sb[:, bass.ts(d, M)],
                             rhs=xt[:, bass.ts(d, SBK)], start=st, stop=sp)
            nc.tensor.matmul(k_ps[:], lhsT=wk_sb[:, bass.ts(d, M)],
                             rhs=xt[:, bass.ts(d, SBK)], start=st, stop=sp)
            nc.tensor.matmul(v_ps[:], lhsT=wv_sb[:, bass.ts(d, M)],
                             rhs=xt[:, bass.ts(d, SBK)], start=st, stop=sp)
        # evictions: qA/qB into zero-padded halves + k on ScalarE (idle
        # during phase 1), v on DVE, then PE-transpose into v_aug slots.
        nc.scalar.copy(qpad[0:64, 2 * sb * SBK:(2 * sb + 1) * SBK],
                       q_ps[0:64, :])
        nc.scalar.copy(qpad[64:128, (2 * sb + 1) * SBK:(2 * sb + 2) * SBK],
                       q_ps[64:128, :])
        nc.scalar.copy(kT_sb[:, bass.ts(sb, SBK)], k_ps[:])
        vt = vt_pool.tile([128, SBK], F16)
        nc.vector.tensor_copy(vt[:], v_ps[:])
        for t in range(SBK // JBK):
            jt = sb * (SBK // JBK) + t
            tp = psB.tile([128, JBK], F16, tag="po")
            nc.tensor.transpose(tp[:], vt[:, bass.ts(t, JBK)], ident[:])
            nc.vector.tensor_copy(v_aug_r[:, jt, 0::2, :], tp[:])

    def emit_proj(qb):
        """Output-projection partial for query block qb -> DRAM (fp16)."""
        qsl = bass.ts(qb, SBK)
        for ob in range(NDB):
            po = psB.tile([128, SBK], F32, tag="po")
            nc.tensor.matmul(po[:], lhsT=wo_sb[:, bass.ts(ob, 128)],
                             rhs=attnT[:, qsl], start=True, stop=True)
            ot = out_pool.tile([128, SBK], F16)
            nc.vector.tensor_copy(ot[:], po[:])
            nc.sync.dma_start(out=outp[bass.ts(ob, 128), qsl], in_=ot[:])

    def attention(qb, pending_proj=None):
        """Causal attention for query block qb (both heads)."""
        nj = 4 * (qb + 1)
        acc_A = psB.tile([128, SBK], F32, tag="acc")
        acc_B = psB.tile([128, SBK], F32, tag="acc")

        def emit_scores(j):
            r = j - (nj - 4)
            off = 128 * r if r > 0 else 0
            sc = psA.tile([128, 2 * SBK], F32, tag="sc")
            kslice = kT_sb[:, bass.ts(j, JBK)]
            qa = qpad[:, 2 * qb * SBK + off:(2 * qb + 1) * SBK]
            qb_ap = qpad[:, (2 * qb + 1) * SBK + off:(2 * qb + 2) * SBK]
            nc.tensor.matmul(sc[:, off:SBK], lhsT=kslice, rhs=qa,
                             start=True, stop=True)
            nc.tensor.matmul(sc[:, SBK + off:2 * SBK], lhsT=kslice,
                             rhs=qb_ap, start=True, stop=True)
            if r >= 0:
                # additive causal mask on the [128,128] diagonal strip of
                # both heads (one 3D DVE op, pre-exp, on PSUM).
                dlo = 128 * r
                sc3 = bass.AP(tensor=sc.tensor, offset=sc.offset + dlo,
                              ap=[list(sc.ap[0]), [SBK, 2], [1, JBK]])
                m3 = mask_sb[:].rearrange("p (b c) -> p b c", b=2)
                nc.vector.tensor_add(sc3, sc3, m3)
            return sc, off

        cur = emit_scores(0)
        for j in range(nj):
            nxt = emit_scores(j + 1) if j + 1 < nj else None
            if pending_proj is not None and j == 2:
                emit_proj(pending_proj)
                pending_proj = None
            sc, off = cur
            pt = pt_pool.tile([128, 2 * SBK], F32R, tag="pt")
            scale = float(1.0 / np.sqrt(HD))
            if off == 0:
                nc.scalar.activation(pt[:], sc[:],
                                     mybir.ActivationFunctionType.Exp,
                                     scale=scale)
            else:
                w = SBK - off
                sc2 = bass.AP(tensor=sc.tensor, offset=sc.offset + off,
                              ap=[list(sc.ap[0]), [SBK, 2], [1, w]])
                pt2 = bass.AP(tensor=pt.tensor, offset=pt.offset + off,
                              ap=[list(pt.ap[0]), [SBK, 2], [1, w]])
                nc.scalar.activation(pt2, sc2,
                                     mybir.ActivationFunctionType.Exp,
                                     scale=scale)
            st, sp = j == 0, j == nj - 1
            vb = j * VW
            nc.tensor.matmul(acc_A[:, off:SBK],
                             lhsT=v_aug[:, vb:vb + 128],
                             rhs=pt[:, off:SBK], start=st, stop=sp)
            nc.tensor.matmul(acc_B[:, off:SBK],
                             lhsT=v_aug[:, vb + HD:vb + VW],
                             rhs=pt[:, SBK + off:2 * SBK],
                             start=st, stop=sp)
            cur = nxt
        if pending_proj is not None:
            emit_proj(pending_proj)
        return acc_A, acc_B

    def normalize(qb, acc_A, acc_B):
        """attnT = acc_out / l. Head A: out rows 0:64, l rows 64:128;
        head B flipped. Cross-partition l moves on ScalarE Copy, then one
        DVE reciprocal + two DVE multiplies."""
        qsl = bass.ts(qb, SBK)
        lt = small.tile([128, SBK], F32, tag="lt")
        nc.scalar.copy(lt[0:64, :], acc_A[HD:2 * HD, :])
        nc.scalar.copy(lt[64:128, :], acc_B[0:HD, :])
        li = small.tile([128, SBK], F32, tag="li")
        nc.vector.reciprocal_approx_fast(out=li[:], in_=lt[:])
        nc.vector.tensor_mul(attnT[0:64, qsl], acc_A[0:HD, :], li[0:64, :])
        nc.vector.tensor_mul(attnT[64:128, qsl], acc_B[HD:2 * HD, :],
                             li[64:128, :])

    # ---- emission ----
    for sb in range(NSB):
        phase1(sb)
        if sb == 0:
            _late_consts()
    for qb in range(NSB):
        accs = attention(qb, pending_proj=qb - 1 if qb > 0 else None)
        normalize(qb, *accs)
    emit_proj(NSB - 1)


def _host_prep(x, Wq, Wk, Wv, Wo):
    xT = np.ascontiguousarray(x.reshape(S, D).T).astype(np.float16)
    jj = np.arange(JBK)[:, None]
    qq = np.arange(JBK)[None, :]
    tri = np.where(jj <= qq, np.float32(0.0), np.float32(NEG))
    nmask = np.concatenate([tri, tri], axis=1)
    in_maps = []
    for c in range(NCORES):
        sl = slice(c * M, (c + 1) * M)
        in_maps.append({
            "xT": xT,
            "wq": np.ascontiguousarray(Wq[sl, :].T).astype(np.float16),
            "wk": np.ascontiguousarray(Wk[sl, :].T).astype(np.float16),
            "wv": np.ascontiguousarray(Wv[sl, :].T).astype(np.float16),
            "wo": np.ascontiguousarray(Wo[:, sl].T).astype(np.float16),
            "ones": np.ones((128, HD), dtype=np.float32),
            "nmask": np.ascontiguousarray(nmask),
        })
    return in_maps


def _run(inputs, trace=False):
    x = np.asarray(inputs["x"], dtype=np.float32)
    Wq = np.asarray(inputs["Wq"], dtype=np.float32)
    Wk = np.asarray(inputs["Wk"], dtype=np.float32)
    Wv = np.asarray(inputs["Wv"], dtype=np.float32)
    Wo = np.asarray(inputs["Wo"], dtype=np.float32)

    if "nc" not in _CACHE:
        _CACHE["nc"] = _build_nc()
    nc = _CACHE["nc"]

    in_maps = _host_prep(x, Wq, Wk, Wv, Wo)
    res = bass_utils.run_bass_kernel_spmd(
        nc, in_maps, core_ids=list(range(NCORES)), trace=trace)
    partial = np.zeros((D, S), dtype=np.float32)
    for c in range(NCORES):
        partial += res.results[c]["outp"].astype(np.float32)
    out = partial.T.astype(np.float32).reshape(B, S, D)
    return out, res


def kernel(x, mask, Wq, Wk, Wv, Wo):
    mask = np.asarray(mask)
    causal = np.tril(np.ones((S, S), dtype=bool))
    if mask.reshape(S, S).shape == causal.shape and bool(
            np.array_equal(mask.reshape(S, S), causal)):
        out, _ = _run({"x": x, "Wq": Wq, "Wk": Wk, "Wv": Wv, "Wo": Wo})
        return out
    # safety net for a non-causal mask: exact numpy fallback
    return _numpy_ref(np.asarray(x, np.float32), mask,
                      np.asarray(Wq, np.float32), np.asarray(Wk, np.float32),
                      np.asarray(Wv, np.float32), np.asarray(Wo, np.float32))


def _numpy_ref(x, mask, Wq, Wk, Wv, Wo):
    xf = x.reshape(S, D)
    q = xf @ Wq.T
    k = xf @ Wk.T
    v = xf @ Wv.T
    m2 = mask.reshape(S, S)
    o = np.empty((S, D), dtype=np.float32)
    for h in range(H):
        hs = slice(h * HD, (h + 1) * HD)
        sc = (q[:, hs] @ k[:, hs].T) / np.sqrt(np.float32(HD))
        sc = np.where(m2, sc, np.float32(-1e9))
        sc -= sc.max(axis=-1, keepdims=True)
        p = np.exp(sc)
        p /= p.sum(axis=-1, keepdims=True)
        o[:, hs] = p @ v[:, hs]
    return (o @ Wo.T).astype(np.float32).reshape(B, S, D)


# revision 13
# speedup vs baseline: 1.1673x; 1.0401x over previous
"""Multi-head causal attention (B=1, S=4096, D=1024, H=16, HD=64) on 8
Trainium2 NeuronCores.

Sharding: head-parallel - 16 heads / 8 cores = 2 heads per core (one
128-channel slice of the QKV/output projections per core).

v2 design notes (vs the f32r baseline at ~310us):
  * All matmul operands are fp16 (psum stays f32). fp16 keeps 1 cyc/row
    streaming, halves LDWEIGHTS (FWL), halves SBUF/DMA bytes, and avoids
    the f32r 4-cyc/row penalty on narrow diagonal tiles. Accuracy budget
    (rel tol 2e-2) has ~20x margin at fp16.
  * Scores use FULL-ARRAY (128-row) stationary kT tiles: trace analysis
    showed full-row LDWEIGHTS pulls ahead under in-flight matmuls
    (phase-1 chains ran at 227ns/MM = stream rate) while the baseline's
    64-row strip LDWs cannot and serialize (~600ns/pair). The two heads
    are separated by ZERO-PADDING the moving Q operand instead: qpad
    holds [qA;0] and [0;qB] blocks so one kT_j stationary serves both
    heads with plain full-array matmuls.
  * Causal masking is additive (-1e5) on the PSUM scores via DVE before
    the exp, removing the GpSimd mask-multiply from the pt->PV critical
    path.
  * Softmax denominator: v_aug = [V_A | ones | V_B] rider (as baseline);
    normalization is DVE reciprocal + multiply (no Ln/Exp table games),
    with two ScalarE Copy ops for the cross-partition l moves.
  * Output projection is interleaved per query-block into the next
    block's attention, evicted on GpSimd, written as fp16 partials.
  * Emission software-pipelines scores(j+1) ahead of PV(j) so the PE
    queue always has independent work while ScalarE runs exp (ScalarE is
    within ~5% of TensorE here; exp is ~110us/core of irreducible work).
"""

import os
import sys

import numpy as np

for _p in ("/opt/trn_rl_repo", "/root/.axon_site/_ro/trn_rl_repo"):
    if os.path.isdir(_p) and _p not in sys.path:
        sys.path.insert(0, _p)

from contextlib import ExitStack

import concourse.bass as bass
import concourse.tile as tile
from concourse import bacc, bass_utils, mybir
from concourse.masks import make_identity

# Problem shape (hardcoded per the harness contract).
B, S, D, H = 1, 4096, 1024, 16
HD = D // H          # 64
NCORES = 8
HPC = H // NCORES    # 2 heads per core
M = HPC * HD         # 128 channels per core
SBK = 512            # query/sequence block size
NSB = S // SBK       # 8
DBK = 128            # d block size
NDB = D // DBK       # 8
JBK = 128            # key block size
NJT = S // JBK       # 32 j-tiles
VW = 3 * HD          # v_aug row width per j-tile: [V_A | ones | V_B]
NEG = -1.0e5         # additive causal mask value (pre-softmax)

F32 = mybir.dt.float32
F16 = mybir.dt.float16
F32R = mybir.dt.float32r

_CACHE = {}


def _build_nc():
    nc = bacc.Bacc("TRN2", target_bir_lowering=False, debug=False,
                   num_devices=NCORES)

    xT = nc.dram_tensor("xT", [D, S], F16, kind="ExternalInput").ap()
    wq = nc.dram_tensor("wq", [D, M], F16, kind="ExternalInput").ap()
    wk = nc.dram_tensor("wk", [D, M], F16, kind="ExternalInput").ap()
    wv = nc.dram_tensor("wv", [D, M], F16, kind="ExternalInput").ap()
    wo = nc.dram_tensor("wo", [M, D], F16, kind="ExternalInput").ap()
    ones = nc.dram_tensor("ones", [128, HD], F16, kind="ExternalInput").ap()
    nmask = nc.dram_tensor("nmask", [JBK, 2 * JBK], F32,
                           kind="ExternalInput").ap()
    outp = nc.dram_tensor("outp", [D, S], F16, kind="ExternalOutput").ap()

    with tile.TileContext(nc) as tc:
        with ExitStack() as ctx:
            _emit(ctx, tc, nc, xT, wq, wk, wv, wo, ones, nmask, outp)
    nc.compile()
    return nc


def _emit(ctx, tc, nc, xT, wq, wk, wv, wo, ones, nmask, outp):
    const = ctx.enter_context(tc.tile_pool(name="const", bufs=1))
    persist = ctx.enter_context(tc.tile_pool(name="persist", bufs=1))
    xt_pool = ctx.enter_context(tc.tile_pool(name="xt", bufs=3))
    vt_pool = ctx.enter_context(tc.tile_pool(name="vt", bufs=2))
    pt_pool = ctx.enter_context(tc.tile_pool(name="pt", bufs=4))
    out_pool = ctx.enter_context(tc.tile_pool(name="outt", bufs=4))
    small = ctx.enter_context(tc.tile_pool(name="small", bufs=2))
    # PSUM budget (16KB/partition = 8 banks):
    #   psA tag "sc":  [128,1024] f32 = 4KB x2 bufs = 8KB (scores; phase1 q/k)
    #   psB tag "acc": [128,512]  f32 = 2KB x2 bufs = 4KB (acc A/B; phase1 v)
    #   psB tag "po":  [128,512]  f32 = 2KB x2 bufs = 4KB (proj out)
    psA = ctx.enter_context(tc.tile_pool(name="psA", bufs=2, space="PSUM"))
    psB = ctx.enter_context(tc.tile_pool(name="psB", bufs=2, space="PSUM"))

    # ---- constants / persistent SBUF ----
    ident = const.tile([128, 128], F16)

    wq_sb = const.tile([128, D], F16)    # 8 d-tiles side by side [d, m]
    wk_sb = const.tile([128, D], F16)
    wv_sb = const.tile([128, D], F16)
    wo_sb = const.tile([128, D], F16)    # [m, o]
    mask_sb = const.tile([JBK, 2 * JBK], F32)

    for w_sb, w_dram in ((wq_sb, wq), (wk_sb, wk), (wv_sb, wv)):
        w_r = w_dram.rearrange("(d p) m -> p d m", p=DBK)
        w_sb_r = w_sb[:].rearrange("p (d m) -> p d m", d=NDB)
        nc.sync.dma_start(out=w_sb_r, in_=w_r)

    def _late_consts():
        for c in range(4):
            nc.sync.dma_start(out=wo_sb[:, bass.ts(c, 256)],
                              in_=wo[:, bass.ts(c, 256)])
        nc.sync.dma_start(out=mask_sb[:], in_=nmask[:])

    # qpad: per query-block layout [qA(512) ; zeros] | [zeros ; qB(512)]
    qpad = persist.tile([128, 2 * S], F16)
    kT_sb = persist.tile([128, S], F16)
    v_aug = persist.tile([128, NJT * VW], F16)
    attnT = persist.tile([128, S], F16)

    # the first xt DMA must be the first GpSimd op so phase 1 starts
    # immediately; ident/memset follow it in the GpSimd queue.
    xt0 = xt_pool.tile([128, NDB * SBK], F16, tag="xt")
    xT_r0 = xT.rearrange("(d p) s -> p d s", p=DBK)
    nc.gpsimd.dma_start(out=xt0[:].rearrange("p (d s) -> p d s", d=NDB),
                        in_=xT_r0[:, :, 0:SBK])
    make_identity(nc, ident)
    nc.vector.memset(qpad[:], 0.0)

    # v_aug per j-tile: [V_A | ones | V_B]; head A reads cols 0:128
    # (denominator in acc rows 64:128), head B reads cols 64:192
    # (denominator in acc rows 0:64). ones filled by one broadcast DMA.
    v_aug_r = v_aug[:].rearrange("p (t c w) -> p t c w", c=3, w=HD)
    ones_bcast = bass.AP(
        tensor=ones.tensor, offset=0,
        ap=[[HD, 128], [0, NJT], [1, HD]])
    nc.sync.dma_start(out=v_aug_r[:, :, 1, :], in_=ones_bcast)

    def phase1(sb, xt=None):
        """QKV projections for s-block sb (512 sequence positions)."""
        q_ps = psA.tile([128, SBK], F32, tag="sc")
        k_ps = psA.tile([128, SBK], F32, tag="sc")
        v_ps = psB.tile([128, SBK], F32, tag="acc")
        if xt is None:
            xt = xt_pool.tile([128, NDB * SBK], F16, tag="xt")
            xt_r = xt[:].rearrange("p (d s) -> p d s", d=NDB)
            xT_r = xT.rearrange("(d p) s -> p d s", p=DBK)
            nc.gpsimd.dma_start(out=xt_r,
                                in_=xT_r[:, :, sb * SBK:(sb + 1) * SBK])
        for d in range(NDB):
            st, sp = d == 0, d == NDB - 1
            nc.tensor.matmul(q_ps[:], lhsT=wq_sb[:, bass.ts(d, M)],
                             rhs=xt[:, bass.ts(d, SBK)], start=st, stop=sp)
            nc.tensor.matmul(k_ps[:], lhsT=wk_sb[:, bass.ts(d, M)],
                             rhs=xt[:, bass.ts(d, SBK)], start=st, stop=sp)
            nc.tensor.matmul(v_ps[:], lhsT=wv_sb[:, bass.ts(d, M)],
                             rhs=xt[:, bass.ts(d, SBK)], start=st, stop=sp)
        # evictions: qA/qB into zero-padded halves + k on ScalarE (idle
        # during phase 1), v on DVE, then PE-transpose into v_aug slots.
        nc.scalar.copy(qpad[0:64, 2 * sb * SBK:(2 * sb + 1) * SBK],
                       q_ps[0:64, :])
        nc.scalar.copy(qpad[64:128, (2 * sb + 1) * SBK:(2 * sb + 2) * SBK],
                       q_ps[64:128, :])
        nc.scalar.copy(kT_sb[:, bass.ts(sb, SBK)], k_ps[:])
        vt = vt_pool.tile([128, SBK], F16)
        nc.vector.tensor_copy(vt[:], v_ps[:])
        for t in range(SBK // JBK):
            jt = sb * (SBK // JBK) + t
            tp = psB.tile([128, JBK], F16, tag="po")
            nc.tensor.transpose(tp[:], vt[:, bass.ts(t, JBK)], ident[:])
            nc.vector.tensor_copy(v_aug_r[:, jt, 0::2, :], tp[:])

    def emit_proj(qb):
        """Output-projection partial for query block qb -> DRAM (fp16)."""
        qsl = bass.ts(qb, SBK)
        for ob in range(NDB):
            po = psB.tile([128, SBK], F32, tag="po")
            nc.tensor.matmul(po[:], lhsT=wo_sb[:, bass.ts(ob, 128)],
                             rhs=attnT[:, qsl], start=True, stop=True)
            ot = out_pool.tile([128, SBK], F16)
            nc.vector.tensor_copy(ot[:], po[:])
            nc.sync.dma_start(out=outp[bass.ts(ob, 128), qsl], in_=ot[:])

    def attention(qb, pending_proj=None):
        """Causal attention for query block qb (both heads)."""
        nj = 4 * (qb + 1)
        acc_A = psB.tile([128, SBK], F32, tag="acc")
        acc_B = psB.tile([128, SBK], F32, tag="acc")

        def emit_scores(j):
            r = j - (nj - 4)
            off = 128 * r if r > 0 else 0
            sc = psA.tile([128, 2 * SBK], F32, tag="sc")
            kslice = kT_sb[:, bass.ts(j, JBK)]
            qa = qpad[:, 2 * qb * SBK + off:(2 * qb + 1) * SBK]
            qb_ap = qpad[:, (2 * qb + 1) * SBK + off:(2 * qb + 2) * SBK]
            nc.tensor.matmul(sc[:, off:SBK], lhsT=kslice, rhs=qa,
                             start=True, stop=True)
            nc.tensor.matmul(sc[:, SBK + off:2 * SBK], lhsT=kslice,
                             rhs=qb_ap, start=True, stop=True)
            if r >= 0:
                # additive causal mask on the [128,128] diagonal strip of
                # both heads (one 3D DVE op, pre-exp, on PSUM).
                dlo = 128 * r
                sc3 = bass.AP(tensor=sc.tensor, offset=sc.offset + dlo,
                              ap=[list(sc.ap[0]), [SBK, 2], [1, JBK]])
                m3 = mask_sb[:].rearrange("p (b c) -> p b c", b=2)
                nc.vector.tensor_add(sc3, sc3, m3)
            return sc, off

        cur = emit_scores(0)
        for j in range(nj):
            nxt = emit_scores(j + 1) if j + 1 < nj else None
            if pending_proj is not None and j == 2:
                emit_proj(pending_proj)
                pending_proj = None
            sc, off = cur
            pt = pt_pool.tile([128, 2 * SBK], F16, tag="pt")
            scale = float(1.0 / np.sqrt(HD))
            if off == 0:
                nc.scalar.activation(pt[:], sc[:],
                                     mybir.ActivationFunctionType.Exp,
                                     scale=scale)
            else:
                w = SBK - off
                sc2 = bass.AP(tensor=sc.tensor, offset=sc.offset + off,
                              ap=[list(sc.ap[0]), [SBK, 2], [1, w]])
                pt2 = bass.AP(tensor=pt.tensor, offset=pt.offset + off,
                              ap=[list(pt.ap[0]), [SBK, 2], [1, w]])
                nc.scalar.activation(pt2, sc2,
                                     mybir.ActivationFunctionType.Exp,
                                     scale=scale)
            st, sp = j == 0, j == nj - 1
            vb = j * VW
            nc.tensor.matmul(acc_A[:, off:SBK],
                             lhsT=v_aug[:, vb:vb + 128],
                             rhs=pt[:, off:SBK], start=st, stop=sp)
            nc.tensor.matmul(acc_B[:, off:SBK],
                             lhsT=v_aug[:, vb + HD:vb + VW],
                             rhs=pt[:, SBK + off:2 * SBK],
                             start=st, stop=sp)
            cur = nxt
        if pending_proj is not None:
            emit_proj(pending_proj)
        return acc_A, acc_B

    def normalize(qb, acc_A, acc_B):
        """attnT = acc_out / l. Head A: out rows 0:64, l rows 64:128;
        head B flipped. Cross-partition l moves on ScalarE Copy, then one
        DVE reciprocal + two DVE multiplies."""
        qsl = bass.ts(qb, SBK)
        lt = small.tile([128, SBK], F32, tag="lt")
        nc.scalar.copy(lt[0:64, :], acc_A[HD:2 * HD, :])
        nc.scalar.copy(lt[64:128, :], acc_B[0:HD, :])
        li = small.tile([128, SBK], F32, tag="li")
        nc.vector.reciprocal_approx_fast(out=li[:], in_=lt[:])
        nc.vector.tensor_mul(attnT[0:64, qsl], acc_A[0:HD, :], li[0:64, :])
        nc.vector.tensor_mul(attnT[64:128, qsl], acc_B[HD:2 * HD, :],
                             li[64:128, :])

    # ---- emission ----
    for sb in range(NSB):
        phase1(sb, xt=xt0 if sb == 0 else None)
        if sb == 0:
            _late_consts()
    for qb in range(NSB):
        accs = attention(qb, pending_proj=qb - 1 if qb > 0 else None)
        normalize(qb, *accs)
    emit_proj(NSB - 1)


def _host_prep(x, Wq, Wk, Wv, Wo):
    xT = np.ascontiguousarray(x.reshape(S, D).T).astype(np.float16)
    jj = np.arange(JBK)[:, None]
    qq = np.arange(JBK)[None, :]
    tri = np.where(jj <= qq, np.float32(0.0), np.float32(NEG))
    nmask = np.concatenate([tri, tri], axis=1)
    in_maps = []
    for c in range(NCORES):
        sl = slice(c * M, (c + 1) * M)
        in_maps.append({
            "xT": xT,
            "wq": np.ascontiguousarray(Wq[sl, :].T).astype(np.float16),
            "wk": np.ascontiguousarray(Wk[sl, :].T).astype(np.float16),
            "wv": np.ascontiguousarray(Wv[sl, :].T).astype(np.float16),
            "wo": np.ascontiguousarray(Wo[:, sl].T).astype(np.float16),
            "ones": np.ones((128, HD), dtype=np.float16),
            "nmask": np.ascontiguousarray(nmask),
        })
    return in_maps


def _run(inputs, trace=False):
    x = np.asarray(inputs["x"], dtype=np.float32)
    Wq = np.asarray(inputs["Wq"], dtype=np.float32)
    Wk = np.asarray(inputs["Wk"], dtype=np.float32)
    Wv = np.asarray(inputs["Wv"], dtype=np.float32)
    Wo = np.asarray(inputs["Wo"], dtype=np.float32)

    if "nc" not in _CACHE:
        _CACHE["nc"] = _build_nc()
    nc = _CACHE["nc"]

    in_maps = _host_prep(x, Wq, Wk, Wv, Wo)
    res = bass_utils.run_bass_kernel_spmd(
        nc, in_maps, core_ids=list(range(NCORES)), trace=trace)
    partial = np.zeros((D, S), dtype=np.float32)
    for c in range(NCORES):
        partial += res.results[c]["outp"].astype(np.float32)
    out = partial.T.astype(np.float32).reshape(B, S, D)
    return out, res


def kernel(x, mask, Wq, Wk, Wv, Wo):
    mask = np.asarray(mask)
    causal = np.tril(np.ones((S, S), dtype=bool))
    if mask.reshape(S, S).shape == causal.shape and bool(
            np.array_equal(mask.reshape(S, S), causal)):
        out, _ = _run({"x": x, "Wq": Wq, "Wk": Wk, "Wv": Wv, "Wo": Wo})
        return out
    # safety net for a non-causal mask: exact numpy fallback
    return _numpy_ref(np.asarray(x, np.float32), mask,
                      np.asarray(Wq, np.float32), np.asarray(Wk, np.float32),
                      np.asarray(Wv, np.float32), np.asarray(Wo, np.float32))


def _numpy_ref(x, mask, Wq, Wk, Wv, Wo):
    xf = x.reshape(S, D)
    q = xf @ Wq.T
    k = xf @ Wk.T
    v = xf @ Wv.T
    m2 = mask.reshape(S, S)
    o = np.empty((S, D), dtype=np.float32)
    for h in range(H):
        hs = slice(h * HD, (h + 1) * HD)
        sc = (q[:, hs] @ k[:, hs].T) / np.sqrt(np.float32(HD))
        sc = np.where(m2, sc, np.float32(-1e9))
        sc -= sc.max(axis=-1, keepdims=True)
        p = np.exp(sc)
        p /= p.sum(axis=-1, keepdims=True)
        o[:, hs] = p @ v[:, hs]
    return (o @ Wo.T).astype(np.float32).reshape(B, S, D)


# revision 16
# speedup vs baseline: 1.3416x; 1.1493x over previous
"""Multi-head causal attention (B=1, S=4096, D=1024, H=16, HD=64) on 8
Trainium2 NeuronCores.

Sharding: head-parallel - 16 heads / 8 cores = 2 heads per core (one
128-channel slice of the QKV/output projections per core).

v2 design notes (vs the f32r baseline at ~310us):
  * All matmul operands are fp16 (psum stays f32). fp16 keeps 1 cyc/row
    streaming, halves LDWEIGHTS (FWL), halves SBUF/DMA bytes, and avoids
    the f32r 4-cyc/row penalty on narrow diagonal tiles. Accuracy budget
    (rel tol 2e-2) has ~20x margin at fp16.
  * Scores use FULL-ARRAY (128-row) stationary kT tiles: trace analysis
    showed full-row LDWEIGHTS pulls ahead under in-flight matmuls
    (phase-1 chains ran at 227ns/MM = stream rate) while the baseline's
    64-row strip LDWs cannot and serialize (~600ns/pair). The two heads
    are separated by ZERO-PADDING the moving Q operand instead: qpad
    holds [qA;0] and [0;qB] blocks so one kT_j stationary serves both
    heads with plain full-array matmuls.
  * Causal masking is additive (-1e5) on the PSUM scores via DVE before
    the exp, removing the GpSimd mask-multiply from the pt->PV critical
    path.
  * Softmax denominator: v_aug = [V_A | ones | V_B] rider (as baseline);
    normalization is DVE reciprocal + multiply (no Ln/Exp table games),
    with two ScalarE Copy ops for the cross-partition l moves.
  * Output projection is interleaved per query-block into the next
    block's attention, evicted on GpSimd, written as fp16 partials.
  * Emission software-pipelines scores(j+1) ahead of PV(j) so the PE
    queue always has independent work while ScalarE runs exp (ScalarE is
    within ~5% of TensorE here; exp is ~110us/core of irreducible work).
"""

import os
import sys

import numpy as np

for _p in ("/opt/trn_rl_repo", "/root/.axon_site/_ro/trn_rl_repo"):
    if os.path.isdir(_p) and _p not in sys.path:
        sys.path.insert(0, _p)

from contextlib import ExitStack

import concourse.bass as bass
import concourse.tile as tile
from concourse import bacc, bass_utils, mybir
from concourse.masks import make_identity

# Problem shape (hardcoded per the harness contract).
B, S, D, H = 1, 4096, 1024, 16
HD = D // H          # 64
NCORES = 8
HPC = H // NCORES    # 2 heads per core
M = HPC * HD         # 128 channels per core
SBK = 512            # query/sequence block size
NSB = S // SBK       # 8
DBK = 128            # d block size
NDB = D // DBK       # 8
JBK = 128            # key block size
NJT = S // JBK       # 32 j-tiles
VW = 3 * HD          # v_aug row width per j-tile: [V_A | ones | V_B]
NEG = -1.0e5         # additive causal mask value (pre-softmax)

F32 = mybir.dt.float32
F16 = mybir.dt.float16
F32R = mybir.dt.float32r

_CACHE = {}


def _build_nc():
    nc = bacc.Bacc("TRN2", target_bir_lowering=False, debug=False,
                   num_devices=NCORES)

    xT = nc.dram_tensor("xT", [D, S], F16, kind="ExternalInput").ap()
    wq = nc.dram_tensor("wq", [D, M], F16, kind="ExternalInput").ap()
    wk = nc.dram_tensor("wk", [D, M], F16, kind="ExternalInput").ap()
    wv = nc.dram_tensor("wv", [D, M], F16, kind="ExternalInput").ap()
    wo = nc.dram_tensor("wo", [M, D], F16, kind="ExternalInput").ap()
    ones = nc.dram_tensor("ones", [128, HD], F16, kind="ExternalInput").ap()
    nmask = nc.dram_tensor("nmask", [JBK, 2 * JBK], F32,
                           kind="ExternalInput").ap()
    outp = nc.dram_tensor("outp", [D, S], F16, kind="ExternalOutput").ap()

    with tile.TileContext(nc) as tc:
        with ExitStack() as ctx:
            _emit(ctx, tc, nc, xT, wq, wk, wv, wo, ones, nmask, outp)
    nc.compile()
    return nc


def _emit(ctx, tc, nc, xT, wq, wk, wv, wo, ones, nmask, outp):
    const = ctx.enter_context(tc.tile_pool(name="const", bufs=1))
    persist = ctx.enter_context(tc.tile_pool(name="persist", bufs=1))
    xt_pool = ctx.enter_context(tc.tile_pool(name="xt", bufs=3))
    vt_pool = ctx.enter_context(tc.tile_pool(name="vt", bufs=2))
    pt_pool = ctx.enter_context(tc.tile_pool(name="pt", bufs=4))
    out_pool = ctx.enter_context(tc.tile_pool(name="outt", bufs=4))
    small = ctx.enter_context(tc.tile_pool(name="small", bufs=2))
    # PSUM budget (16KB/partition = 8 banks):
    #   psA tag "sc":  [128,1024] f32 = 4KB x2 bufs = 8KB (scores; phase1 q/k)
    #   psB tag "acc": [128,512]  f32 = 2KB x2 bufs = 4KB (acc A/B; phase1 v)
    #   psB tag "po":  [128,512]  f32 = 2KB x2 bufs = 4KB (proj out)
    psA = ctx.enter_context(tc.tile_pool(name="psA", bufs=2, space="PSUM"))
    psB = ctx.enter_context(tc.tile_pool(name="psB", bufs=2, space="PSUM"))

    # ---- constants / persistent SBUF ----
    ident = const.tile([128, 128], F16)

    wq_sb = const.tile([128, D], F16)    # 8 d-tiles side by side [d, m]
    wk_sb = const.tile([128, D], F16)
    wv_sb = const.tile([128, D], F16)
    wo_sb = const.tile([128, D], F16)    # [m, o]
    mask_sb = const.tile([JBK, 2 * JBK], F32)

    for w_sb, w_dram in ((wq_sb, wq), (wk_sb, wk), (wv_sb, wv)):
        w_r = w_dram.rearrange("(d p) m -> p d m", p=DBK)
        w_sb_r = w_sb[:].rearrange("p (d m) -> p d m", d=NDB)
        nc.sync.dma_start(out=w_sb_r[:, 0:4, :], in_=w_r[:, 0:4, :])
        nc.sync.dma_start(out=w_sb_r[:, 4:8, :], in_=w_r[:, 4:8, :])

    def _late_consts():
        for c in range(4):
            nc.sync.dma_start(out=wo_sb[:, bass.ts(c, 256)],
                              in_=wo[:, bass.ts(c, 256)])
        nc.sync.dma_start(out=mask_sb[:], in_=nmask[:])

    # qpad: per query-block layout [qA(512) ; zeros] | [zeros ; qB(512)]
    qpad = persist.tile([128, 2 * S], F16)
    kT_sb = persist.tile([128, S], F16)
    v_aug = persist.tile([128, NJT * VW], F16)
    attnT = persist.tile([128, S], F16)

    # the first xt DMA must be the first GpSimd op so phase 1 starts
    # immediately; ident/memset follow it in the GpSimd queue.
    xt0 = xt_pool.tile([128, NDB * SBK], F16, tag="xt")
    xT_r0 = xT.rearrange("(d p) s -> p d s", p=DBK)
    nc.gpsimd.dma_start(out=xt0[:].rearrange("p (d s) -> p d s", d=NDB),
                        in_=xT_r0[:, :, 0:SBK])
    make_identity(nc, ident)
    nc.vector.memset(qpad[:], 0.0)

    # v_aug per j-tile: [V_A | ones | V_B]; head A reads cols 0:128
    # (denominator in acc rows 64:128), head B reads cols 64:192
    # (denominator in acc rows 0:64). ones filled by one broadcast DMA.
    v_aug_r = v_aug[:].rearrange("p (t c w) -> p t c w", c=3, w=HD)
    ones_bcast = bass.AP(
        tensor=ones.tensor, offset=0,
        ap=[[HD, 128], [0, NJT], [1, HD]])
    nc.sync.dma_start(out=v_aug_r[:, :, 1, :], in_=ones_bcast)

    def phase1_first(xt):
        """QKV projections for s-block 0, run before attention starts."""
        q_ps = psA.tile([128, SBK], F32, tag="sc")
        k_ps = psA.tile([128, SBK], F32, tag="sc")
        v_ps = psB.tile([128, SBK], F32, tag="acc")
        for d in range(NDB):
            st, sp = d == 0, d == NDB - 1
            nc.tensor.matmul(q_ps[:], lhsT=wq_sb[:, bass.ts(d, M)],
                             rhs=xt[:, bass.ts(d, SBK)], start=st, stop=sp)
            nc.tensor.matmul(k_ps[:], lhsT=wk_sb[:, bass.ts(d, M)],
                             rhs=xt[:, bass.ts(d, SBK)], start=st, stop=sp)
            nc.tensor.matmul(v_ps[:], lhsT=wv_sb[:, bass.ts(d, M)],
                             rhs=xt[:, bass.ts(d, SBK)], start=st, stop=sp)
        nc.scalar.copy(qpad[0:64, 0:SBK], q_ps[0:64, :])
        nc.scalar.copy(qpad[64:128, SBK:2 * SBK], q_ps[64:128, :])
        nc.scalar.copy(kT_sb[:, 0:SBK], k_ps[:])
        vt = vt_pool.tile([128, SBK], F16, tag="vt")
        nc.vector.tensor_copy(vt[:], v_ps[:])
        for t in range(SBK // JBK):
            tp = psB.tile([128, JBK], F16, tag="po")
            nc.tensor.transpose(tp[:], vt[:, bass.ts(t, JBK)], ident[:])
            nc.vector.tensor_copy(v_aug_r[:, t, 0::2, :], tp[:])

    def make_phase1_tasks(sb):
        """Phase-1 work for s-block sb as PE side-tasks (run interleaved
        into the previous attention block; evictions on DVE)."""
        xt = xt_pool.tile([128, NDB * SBK], F16, tag="xt", name=f"xt{sb}")
        xT_r = xT.rearrange("(d p) s -> p d s", p=DBK)
        nc.gpsimd.dma_start(out=xt[:].rearrange("p (d s) -> p d s", d=NDB),
                            in_=xT_r[:, :, sb * SBK:(sb + 1) * SBK])
        st_ = {}

        def chain_half(kind, w_sb, lo):
            def t():
                if lo == 0:
                    st_[kind] = psB.tile([128, SBK], F32, tag="po",
                                         name=f"p1{kind}{sb}")
                ps = st_[kind]
                for d in range(lo, lo + 4):
                    nc.tensor.matmul(ps[:], lhsT=w_sb[:, bass.ts(d, M)],
                                     rhs=xt[:, bass.ts(d, SBK)],
                                     start=d == 0, stop=d == NDB - 1)
                if lo == 4:
                    if kind == "q":
                        nc.vector.tensor_copy(
                            qpad[0:64, 2 * sb * SBK:(2 * sb + 1) * SBK],
                            ps[0:64, :])
                        nc.vector.tensor_copy(
                            qpad[64:128, (2 * sb + 1) * SBK:(2 * sb + 2) * SBK],
                            ps[64:128, :])
                    elif kind == "k":
                        nc.vector.tensor_copy(kT_sb[:, bass.ts(sb, SBK)],
                                              ps[:])
                    else:
                        vt = vt_pool.tile([128, SBK], F16, tag="vt", name=f"vt{sb}")
                        nc.vector.tensor_copy(vt[:], ps[:])
                        st_["vt"] = vt
            return t

        def tp_pair(t0):
            def t():
                vt = st_["vt"]
                for tt in (t0, t0 + 1):
                    tp = psB.tile([128, JBK], F16, tag="po",
                                  name=f"tp{sb}_{tt}")
                    nc.tensor.transpose(tp[:], vt[:, bass.ts(tt, JBK)],
                                        ident[:])
                    nc.vector.tensor_copy(
                        v_aug_r[:, sb * (SBK // JBK) + tt, 0::2, :], tp[:])
            return t

        return ([chain_half(k, w, lo)
                 for k, w in (("q", wq_sb), ("k", wk_sb), ("v", wv_sb))
                 for lo in (0, 4)]
                + [tp_pair(0), tp_pair(2)])

    def make_proj_tasks(qb):
        """Output-projection partial for query block qb (one task per
        128-wide output slice; eviction on DVE, store on Sync)."""
        qsl = bass.ts(qb, SBK)

        def mk(ob):
            def t():
                po = psB.tile([128, SBK], F32, tag="po", name=f"po{qb}_{ob}")
                nc.tensor.matmul(po[:], lhsT=wo_sb[:, bass.ts(ob, 128)],
                                 rhs=attnT[:, qsl], start=True, stop=True)
                ot = out_pool.tile([128, SBK], F16, tag="ot", name=f"ot{qb}_{ob}")
                nc.vector.tensor_copy(ot[:], po[:])
                nc.sync.dma_start(out=outp[bass.ts(ob, 128), qsl], in_=ot[:])
            return t

        return [mk(ob) for ob in range(NDB)]

    def attention(qb, p1q, prq):
        """Causal attention for query block qb (both heads). Pops side
        tasks (phase-1 chains, projections) between the score and PV
        matmuls of each j-iteration so they fill the exp-wait bubble."""
        nj = 4 * (qb + 1)
        acc_A = psB.tile([128, SBK], F32, tag="acc")
        acc_B = psB.tile([128, SBK], F32, tag="acc")

        def emit_scores(j):
            r = j - (nj - 4)
            off = 128 * r if r > 0 else 0
            sc = psA.tile([128, 2 * SBK], F32, tag="sc")
            kslice = kT_sb[:, bass.ts(j, JBK)]
            qa = qpad[:, 2 * qb * SBK + off:(2 * qb + 1) * SBK]
            qb_ap = qpad[:, (2 * qb + 1) * SBK + off:(2 * qb + 2) * SBK]
            nc.tensor.matmul(sc[:, off:SBK], lhsT=kslice, rhs=qa,
                             start=True, stop=True)
            nc.tensor.matmul(sc[:, SBK + off:2 * SBK], lhsT=kslice,
                             rhs=qb_ap, start=True, stop=True)
            if r >= 0:
                # additive causal mask on the [128,128] diagonal strip of
                # both heads (one 3D DVE op, pre-exp, on PSUM).
                dlo = 128 * r
                sc3 = bass.AP(tensor=sc.tensor, offset=sc.offset + dlo,
                              ap=[list(sc.ap[0]), [SBK, 2], [1, JBK]])
                m3 = mask_sb[:].rearrange("p (b c) -> p b c", b=2)
                nc.vector.tensor_add(sc3, sc3, m3)
            return sc, off

        cur = emit_scores(0)
        for j in range(nj):
            nxt = emit_scores(j + 1) if j + 1 < nj else None
            if p1q:
                p1q.popleft()()
            elif prq:
                prq.popleft()()
                if prq:
                    prq.popleft()()
            sc, off = cur
            pt = pt_pool.tile([128, 2 * SBK], F16, tag="pt")
            scale = float(1.0 / np.sqrt(HD))
            if off == 0:
                nc.scalar.activation(pt[:], sc[:],
                                     mybir.ActivationFunctionType.Exp,
                                     scale=scale)
            else:
                w = SBK - off
                sc2 = bass.AP(tensor=sc.tensor, offset=sc.offset + off,
                              ap=[list(sc.ap[0]), [SBK, 2], [1, w]])
                pt2 = bass.AP(tensor=pt.tensor, offset=pt.offset + off,
                              ap=[list(pt.ap[0]), [SBK, 2], [1, w]])
                nc.scalar.activation(pt2, sc2,
                                     mybir.ActivationFunctionType.Exp,
                                     scale=scale)
            st, sp = j == 0, j == nj - 1
            vb = j * VW
            nc.tensor.matmul(acc_A[:, off:SBK],
                             lhsT=v_aug[:, vb:vb + 128],
                             rhs=pt[:, off:SBK], start=st, stop=sp)
            nc.tensor.matmul(acc_B[:, off:SBK],
                             lhsT=v_aug[:, vb + HD:vb + VW],
                             rhs=pt[:, SBK + off:2 * SBK],
                             start=st, stop=sp)
            cur = nxt
        # phase-1 tasks for the next block must complete before it starts
        while p1q:
            p1q.popleft()()
        return acc_A, acc_B

    def normalize(qb, acc_A, acc_B):
        """attnT = acc_out / l. Head A: out rows 0:64, l rows 64:128;
        head B flipped. Cross-partition l moves on ScalarE Copy, then one
        DVE reciprocal + two DVE multiplies."""
        qsl = bass.ts(qb, SBK)
        lt = small.tile([128, SBK], F32, tag="lt")
        nc.scalar.copy(lt[0:64, :], acc_A[HD:2 * HD, :])
        nc.scalar.copy(lt[64:128, :], acc_B[0:HD, :])
        li = small.tile([128, SBK], F32, tag="li")
        nc.vector.reciprocal_approx_fast(out=li[:], in_=lt[:])
        nc.vector.tensor_mul(attnT[0:64, qsl], acc_A[0:HD, :], li[0:64, :])
        nc.vector.tensor_mul(attnT[64:128, qsl], acc_B[HD:2 * HD, :],
                             li[64:128, :])

    # ---- emission ----
    from collections import deque
    phase1_first(xt0)
    _late_consts()
    p1q, prq = deque(), deque()
    for qb in range(NSB):
        if qb + 1 < NSB:
            p1q.extend(make_phase1_tasks(qb + 1))
        accs = attention(qb, p1q, prq)
        normalize(qb, *accs)
        prq.extend(make_proj_tasks(qb))
    while prq:
        prq.popleft()()


def _host_prep(x, Wq, Wk, Wv, Wo):
    xT = np.ascontiguousarray(x.reshape(S, D).T).astype(np.float16)
    jj = np.arange(JBK)[:, None]
    qq = np.arange(JBK)[None, :]
    tri = np.where(jj <= qq, np.float32(0.0), np.float32(NEG))
    nmask = np.concatenate([tri, tri], axis=1)
    in_maps = []
    for c in range(NCORES):
        sl = slice(c * M, (c + 1) * M)
        in_maps.append({
            "xT": xT,
            "wq": np.ascontiguousarray(Wq[sl, :].T).astype(np.float16),
            "wk": np.ascontiguousarray(Wk[sl, :].T).astype(np.float16),
            "wv": np.ascontiguousarray(Wv[sl, :].T).astype(np.float16),
            "wo": np.ascontiguousarray(Wo[:, sl].T).astype(np.float16),
            "ones": np.ones((128, HD), dtype=np.float16),
            "nmask": np.ascontiguousarray(nmask),
        })
    return in_maps


def _run(inputs, trace=False):
    x = np.asarray(inputs["x"], dtype=np.float32)
    Wq = np.asarray(inputs["Wq"], dtype=np.float32)
    Wk = np.asarray(inputs["Wk"], dtype=np.float32)
    Wv = np.asarray(inputs["Wv"], dtype=np.float32)
    Wo = np.asarray(inputs["Wo"], dtype=np.float32)

    if "nc" not in _CACHE:
        _CACHE["nc"] = _build_nc()
    nc = _CACHE["nc"]

    in_maps = _host_prep(x, Wq, Wk, Wv, Wo)
    res = bass_utils.run_bass_kernel_spmd(
        nc, in_maps, core_ids=list(range(NCORES)), trace=trace)
    partial = np.zeros((D, S), dtype=np.float32)
    for c in range(NCORES):
        partial += res.results[c]["outp"].astype(np.float32)
    out = partial.T.astype(np.float32).reshape(B, S, D)
    return out, res


def kernel(x, mask, Wq, Wk, Wv, Wo):
    mask = np.asarray(mask)
    causal = np.tril(np.ones((S, S), dtype=bool))
    if mask.reshape(S, S).shape == causal.shape and bool(
            np.array_equal(mask.reshape(S, S), causal)):
        out, _ = _run({"x": x, "Wq": Wq, "Wk": Wk, "Wv": Wv, "Wo": Wo})
        return out
    # safety net for a non-causal mask: exact numpy fallback
    return _numpy_ref(np.asarray(x, np.float32), mask,
                      np.asarray(Wq, np.float32), np.asarray(Wk, np.float32),
                      np.asarray(Wv, np.float32), np.asarray(Wo, np.float32))


def _numpy_ref(x, mask, Wq, Wk, Wv, Wo):
    xf = x.reshape(S, D)
    q = xf @ Wq.T
    k = xf @ Wk.T
    v = xf @ Wv.T
    m2 = mask.reshape(S, S)
    o = np.empty((S, D), dtype=np.float32)
    for h in range(H):
        hs = slice(h * HD, (h + 1) * HD)
        sc = (q[:, hs] @ k[:, hs].T) / np.sqrt(np.float32(HD))
        sc = np.where(m2, sc, np.float32(-1e9))
        sc -= sc.max(axis=-1, keepdims=True)
        p = np.exp(sc)
        p /= p.sum(axis=-1, keepdims=True)
        o[:, hs] = p @ v[:, hs]
    return (o @ Wo.T).astype(np.float32).reshape(B, S, D)


# revision 17
# speedup vs baseline: 1.3425x; 1.0007x over previous
"""Multi-head causal attention (B=1, S=4096, D=1024, H=16, HD=64) on 8
Trainium2 NeuronCores.

Sharding: head-parallel - 16 heads / 8 cores = 2 heads per core (one
128-channel slice of the QKV/output projections per core).

v2 design notes (vs the f32r baseline at ~310us):
  * All matmul operands are fp16 (psum stays f32). fp16 keeps 1 cyc/row
    streaming, halves LDWEIGHTS (FWL), halves SBUF/DMA bytes, and avoids
    the f32r 4-cyc/row penalty on narrow diagonal tiles. Accuracy budget
    (rel tol 2e-2) has ~20x margin at fp16.
  * Scores use FULL-ARRAY (128-row) stationary kT tiles: trace analysis
    showed full-row LDWEIGHTS pulls ahead under in-flight matmuls
    (phase-1 chains ran at 227ns/MM = stream rate) while the baseline's
    64-row strip LDWs cannot and serialize (~600ns/pair). The two heads
    are separated by ZERO-PADDING the moving Q operand instead: qpad
    holds [qA;0] and [0;qB] blocks so one kT_j stationary serves both
    heads with plain full-array matmuls.
  * Causal masking is additive (-1e5) on the PSUM scores via DVE before
    the exp, removing the GpSimd mask-multiply from the pt->PV critical
    path.
  * Softmax denominator: v_aug = [V_A | ones | V_B] rider (as baseline);
    normalization is DVE reciprocal + multiply (no Ln/Exp table games),
    with two ScalarE Copy ops for the cross-partition l moves.
  * Output projection is interleaved per query-block into the next
    block's attention, evicted on GpSimd, written as fp16 partials.
  * Emission software-pipelines scores(j+1) ahead of PV(j) so the PE
    queue always has independent work while ScalarE runs exp (ScalarE is
    within ~5% of TensorE here; exp is ~110us/core of irreducible work).
"""

import os
import sys

import numpy as np

for _p in ("/opt/trn_rl_repo", "/root/.axon_site/_ro/trn_rl_repo"):
    if os.path.isdir(_p) and _p not in sys.path:
        sys.path.insert(0, _p)

from contextlib import ExitStack

import concourse.bass as bass
import concourse.tile as tile
from concourse import bacc, bass_utils, mybir
from concourse.masks import make_identity

# Problem shape (hardcoded per the harness contract).
B, S, D, H = 1, 4096, 1024, 16
HD = D // H          # 64
NCORES = 8
HPC = H // NCORES    # 2 heads per core
M = HPC * HD         # 128 channels per core
SBK = 512            # query/sequence block size
NSB = S // SBK       # 8
DBK = 128            # d block size
NDB = D // DBK       # 8
JBK = 128            # key block size
NJT = S // JBK       # 32 j-tiles
VW = 3 * HD          # v_aug row width per j-tile: [V_A | ones | V_B]
NEG = -1.0e5         # additive causal mask value (pre-softmax)

F32 = mybir.dt.float32
F16 = mybir.dt.float16
F32R = mybir.dt.float32r

_CACHE = {}


def _build_nc():
    nc = bacc.Bacc("TRN2", target_bir_lowering=False, debug=False,
                   num_devices=NCORES)

    xT = nc.dram_tensor("xT", [D, S], F16, kind="ExternalInput").ap()
    wq = nc.dram_tensor("wq", [D, M], F16, kind="ExternalInput").ap()
    wk = nc.dram_tensor("wk", [D, M], F16, kind="ExternalInput").ap()
    wv = nc.dram_tensor("wv", [D, M], F16, kind="ExternalInput").ap()
    wo = nc.dram_tensor("wo", [M, D], F16, kind="ExternalInput").ap()
    ones = nc.dram_tensor("ones", [128, HD], F16, kind="ExternalInput").ap()
    nmask = nc.dram_tensor("nmask", [JBK, 2 * JBK], F32,
                           kind="ExternalInput").ap()
    outp = nc.dram_tensor("outp", [D, S], F16, kind="ExternalOutput").ap()

    with tile.TileContext(nc) as tc:
        with ExitStack() as ctx:
            _emit(ctx, tc, nc, xT, wq, wk, wv, wo, ones, nmask, outp)
    nc.compile()
    return nc


def _emit(ctx, tc, nc, xT, wq, wk, wv, wo, ones, nmask, outp):
    const = ctx.enter_context(tc.tile_pool(name="const", bufs=1))
    persist = ctx.enter_context(tc.tile_pool(name="persist", bufs=1))
    xt_pool = ctx.enter_context(tc.tile_pool(name="xt", bufs=3))
    vt_pool = ctx.enter_context(tc.tile_pool(name="vt", bufs=2))
    pt_pool = ctx.enter_context(tc.tile_pool(name="pt", bufs=4))
    out_pool = ctx.enter_context(tc.tile_pool(name="outt", bufs=4))
    small = ctx.enter_context(tc.tile_pool(name="small", bufs=2))
    # PSUM budget (16KB/partition = 8 banks):
    #   psA tag "sc":  [128,1024] f32 = 4KB x2 bufs = 8KB (scores; phase1 q/k)
    #   psB tag "acc": [128,512]  f32 = 2KB x2 bufs = 4KB (acc A/B; phase1 v)
    #   psB tag "po":  [128,512]  f32 = 2KB x2 bufs = 4KB (proj out)
    psA = ctx.enter_context(tc.tile_pool(name="psA", bufs=2, space="PSUM"))
    psB = ctx.enter_context(tc.tile_pool(name="psB", bufs=2, space="PSUM"))

    # ---- constants / persistent SBUF ----
    ident = const.tile([128, 128], F16)

    wq_sb = const.tile([128, D], F16)    # 8 d-tiles side by side [d, m]
    wk_sb = const.tile([128, D], F16)
    wv_sb = const.tile([128, D], F16)
    wo_sb = const.tile([128, D], F16)    # [m, o]
    mask_sb = const.tile([JBK, 2 * JBK], F32)

    for w_sb, w_dram in ((wq_sb, wq), (wk_sb, wk), (wv_sb, wv)):
        w_r = w_dram.rearrange("(d p) m -> p d m", p=DBK)
        w_sb_r = w_sb[:].rearrange("p (d m) -> p d m", d=NDB)
        nc.sync.dma_start(out=w_sb_r[:, 0:4, :], in_=w_r[:, 0:4, :])
        nc.sync.dma_start(out=w_sb_r[:, 4:8, :], in_=w_r[:, 4:8, :])

    def _late_consts():
        for c in range(4):
            nc.sync.dma_start(out=wo_sb[:, bass.ts(c, 256)],
                              in_=wo[:, bass.ts(c, 256)])
        nc.sync.dma_start(out=mask_sb[:], in_=nmask[:])

    # qpad: per query-block layout [qA(512) ; zeros] | [zeros ; qB(512)]
    qpad = persist.tile([128, 2 * S], F16)
    kT_sb = persist.tile([128, S], F16)
    v_aug = persist.tile([128, NJT * VW], F16)
    attnT = persist.tile([128, S], F16)

    # the first xt DMA must be the first GpSimd op so phase 1 starts
    # immediately; ident/memset follow it in the GpSimd queue.
    xt0 = xt_pool.tile([128, NDB * SBK], F16, tag="xt")
    xT_r0 = xT.rearrange("(d p) s -> p d s", p=DBK)
    xt0_r = xt0[:].rearrange("p (d s) -> p d s", d=NDB)
    nc.gpsimd.dma_start(out=xt0_r[:, 0:4, :], in_=xT_r0[:, 0:4, 0:SBK])
    nc.gpsimd.dma_start(out=xt0_r[:, 4:8, :], in_=xT_r0[:, 4:8, 0:SBK])
    make_identity(nc, ident)
    nc.vector.memset(qpad[:], 0.0)

    # v_aug per j-tile: [V_A | ones | V_B]; head A reads cols 0:128
    # (denominator in acc rows 64:128), head B reads cols 64:192
    # (denominator in acc rows 0:64). ones filled by one broadcast DMA.
    v_aug_r = v_aug[:].rearrange("p (t c w) -> p t c w", c=3, w=HD)
    ones_bcast = bass.AP(
        tensor=ones.tensor, offset=0,
        ap=[[HD, 128], [0, NJT], [1, HD]])
    nc.sync.dma_start(out=v_aug_r[:, :, 1, :], in_=ones_bcast)

    def phase1_first(xt):
        """QKV projections for s-block 0, run before attention starts."""
        q_ps = psA.tile([128, SBK], F32, tag="sc")
        k_ps = psA.tile([128, SBK], F32, tag="sc")
        v_ps = psB.tile([128, SBK], F32, tag="acc")
        for d in range(NDB):
            st, sp = d == 0, d == NDB - 1
            nc.tensor.matmul(q_ps[:], lhsT=wq_sb[:, bass.ts(d, M)],
                             rhs=xt[:, bass.ts(d, SBK)], start=st, stop=sp)
            nc.tensor.matmul(k_ps[:], lhsT=wk_sb[:, bass.ts(d, M)],
                             rhs=xt[:, bass.ts(d, SBK)], start=st, stop=sp)
            nc.tensor.matmul(v_ps[:], lhsT=wv_sb[:, bass.ts(d, M)],
                             rhs=xt[:, bass.ts(d, SBK)], start=st, stop=sp)
        nc.scalar.copy(qpad[0:64, 0:SBK], q_ps[0:64, :])
        nc.scalar.copy(qpad[64:128, SBK:2 * SBK], q_ps[64:128, :])
        nc.scalar.copy(kT_sb[:, 0:SBK], k_ps[:])
        vt = vt_pool.tile([128, SBK], F16, tag="vt")
        nc.vector.tensor_copy(vt[:], v_ps[:])
        for t in range(SBK // JBK):
            tp = psB.tile([128, JBK], F16, tag="po")
            nc.tensor.transpose(tp[:], vt[:, bass.ts(t, JBK)], ident[:])
            nc.vector.tensor_copy(v_aug_r[:, t, 0::2, :], tp[:])

    def make_phase1_tasks(sb):
        """Phase-1 work for s-block sb as PE side-tasks (run interleaved
        into the previous attention block; evictions on DVE)."""
        xt = xt_pool.tile([128, NDB * SBK], F16, tag="xt", name=f"xt{sb}")
        xT_r = xT.rearrange("(d p) s -> p d s", p=DBK)
        nc.gpsimd.dma_start(out=xt[:].rearrange("p (d s) -> p d s", d=NDB),
                            in_=xT_r[:, :, sb * SBK:(sb + 1) * SBK])
        st_ = {}

        def chain_half(kind, w_sb, lo):
            def t():
                if lo == 0:
                    st_[kind] = psB.tile([128, SBK], F32, tag="po",
                                         name=f"p1{kind}{sb}")
                ps = st_[kind]
                for d in range(lo, lo + 4):
                    nc.tensor.matmul(ps[:], lhsT=w_sb[:, bass.ts(d, M)],
                                     rhs=xt[:, bass.ts(d, SBK)],
                                     start=d == 0, stop=d == NDB - 1)
                if lo == 4:
                    if kind == "q":
                        nc.vector.tensor_copy(
                            qpad[0:64, 2 * sb * SBK:(2 * sb + 1) * SBK],
                            ps[0:64, :])
                        nc.vector.tensor_copy(
                            qpad[64:128, (2 * sb + 1) * SBK:(2 * sb + 2) * SBK],
                            ps[64:128, :])
                    elif kind == "k":
                        nc.vector.tensor_copy(kT_sb[:, bass.ts(sb, SBK)],
                                              ps[:])
                    else:
                        vt = vt_pool.tile([128, SBK], F16, tag="vt", name=f"vt{sb}")
                        nc.vector.tensor_copy(vt[:], ps[:])
                        st_["vt"] = vt
            return t

        def tp_pair(t0):
            def t():
                vt = st_["vt"]
                for tt in (t0, t0 + 1):
                    tp = psB.tile([128, JBK], F16, tag="po",
                                  name=f"tp{sb}_{tt}")
                    nc.tensor.transpose(tp[:], vt[:, bass.ts(tt, JBK)],
                                        ident[:])
                    nc.vector.tensor_copy(
                        v_aug_r[:, sb * (SBK // JBK) + tt, 0::2, :], tp[:])
            return t

        return ([(sb, k, chain_half(k, w, lo))
                 for k, w in (("q", wq_sb), ("k", wk_sb), ("v", wv_sb))
                 for lo in (0, 4)]
                + [(sb, "tp", tp_pair(0)), (sb, "tp", tp_pair(2))])

    def make_proj_tasks(qb, split_evict=False):
        """Output-projection partial for query block qb (one task per
        128-wide output slice; eviction on DVE, store on Sync). With
        split_evict, alternate evictions between ScalarE and DVE (used
        for the final block where both engines are otherwise idle)."""
        qsl = bass.ts(qb, SBK)

        def mk(ob):
            def t():
                po = psB.tile([128, SBK], F32, tag="po", name=f"po{qb}_{ob}")
                nc.tensor.matmul(po[:], lhsT=wo_sb[:, bass.ts(ob, 128)],
                                 rhs=attnT[:, qsl], start=True, stop=True)
                ot = out_pool.tile([128, SBK], F16, tag="ot", name=f"ot{qb}_{ob}")
                if split_evict and ob % 2 == 0:
                    nc.scalar.copy(ot[:], po[:])
                else:
                    nc.vector.tensor_copy(ot[:], po[:])
                nc.sync.dma_start(out=outp[bass.ts(ob, 128), qsl], in_=ot[:])
            return t

        return [mk(ob) for ob in range(NDB)]

    def attention(qb, p1q, prq):
        """Causal attention for query block qb (both heads). Pops side
        tasks (phase-1 chains, projections) between the score and PV
        matmuls of each j-iteration so they fill the exp-wait bubble."""
        nj = 4 * (qb + 1)
        acc_A = psB.tile([128, SBK], F32, tag="acc")
        acc_B = psB.tile([128, SBK], F32, tag="acc")

        def emit_scores(j):
            r = j - (nj - 4)
            off = 128 * r if r > 0 else 0
            sc = psA.tile([128, 2 * SBK], F32, tag="sc")
            kslice = kT_sb[:, bass.ts(j, JBK)]
            qa = qpad[:, 2 * qb * SBK + off:(2 * qb + 1) * SBK]
            qb_ap = qpad[:, (2 * qb + 1) * SBK + off:(2 * qb + 2) * SBK]
            nc.tensor.matmul(sc[:, off:SBK], lhsT=kslice, rhs=qa,
                             start=True, stop=True)
            nc.tensor.matmul(sc[:, SBK + off:2 * SBK], lhsT=kslice,
                             rhs=qb_ap, start=True, stop=True)
            if r >= 0:
                # additive causal mask on the [128,128] diagonal strip of
                # both heads (one 3D DVE op, pre-exp, on PSUM).
                dlo = 128 * r
                sc3 = bass.AP(tensor=sc.tensor, offset=sc.offset + dlo,
                              ap=[list(sc.ap[0]), [SBK, 2], [1, JBK]])
                m3 = mask_sb[:].rearrange("p (b c) -> p b c", b=2)
                nc.vector.tensor_add(sc3, sc3, m3)
            return sc, off

        cur = emit_scores(0)
        for j in range(nj):
            if j + 1 < nj:
                if j + 1 >= nj - 4:
                    # the diagonal j-tiles read kT/v_aug of s-block qb:
                    # force-complete all phase-1 work for sb <= qb first.
                    while p1q and p1q[0][0] <= qb:
                        p1q.popleft()[2]()
                nxt = emit_scores(j + 1)
            else:
                nxt = None
            if p1q:
                p1q.popleft()[2]()
            elif prq:
                prq.popleft()()
                if prq:
                    prq.popleft()()
            sc, off = cur
            pt = pt_pool.tile([128, 2 * SBK], F16, tag="pt")
            scale = float(1.0 / np.sqrt(HD))
            if off == 0:
                nc.scalar.activation(pt[:], sc[:],
                                     mybir.ActivationFunctionType.Exp,
                                     scale=scale)
            else:
                w = SBK - off
                sc2 = bass.AP(tensor=sc.tensor, offset=sc.offset + off,
                              ap=[list(sc.ap[0]), [SBK, 2], [1, w]])
                pt2 = bass.AP(tensor=pt.tensor, offset=pt.offset + off,
                              ap=[list(pt.ap[0]), [SBK, 2], [1, w]])
                nc.scalar.activation(pt2, sc2,
                                     mybir.ActivationFunctionType.Exp,
                                     scale=scale)
            st, sp = j == 0, j == nj - 1
            vb = j * VW
            nc.tensor.matmul(acc_A[:, off:SBK],
                             lhsT=v_aug[:, vb:vb + 128],
                             rhs=pt[:, off:SBK], start=st, stop=sp)
            nc.tensor.matmul(acc_B[:, off:SBK],
                             lhsT=v_aug[:, vb + HD:vb + VW],
                             rhs=pt[:, SBK + off:2 * SBK],
                             start=st, stop=sp)
            cur = nxt
        # the next block's scores need its qpad half: finish the q-chain
        while p1q and p1q[0][0] == qb + 1 and p1q[0][1] == "q":
            p1q.popleft()[2]()
        return acc_A, acc_B

    def normalize(qb, acc_A, acc_B):
        """attnT = acc_out / l. Head A: out rows 0:64, l rows 64:128;
        head B flipped. Cross-partition l moves on ScalarE Copy, then one
        DVE reciprocal + two DVE multiplies."""
        qsl = bass.ts(qb, SBK)
        lt = small.tile([128, SBK], F32, tag="lt")
        nc.scalar.copy(lt[0:64, :], acc_A[HD:2 * HD, :])
        nc.scalar.copy(lt[64:128, :], acc_B[0:HD, :])
        li = small.tile([128, SBK], F32, tag="li")
        nc.vector.reciprocal_approx_fast(out=li[:], in_=lt[:])
        nc.vector.tensor_mul(attnT[0:64, qsl], acc_A[0:HD, :], li[0:64, :])
        nc.vector.tensor_mul(attnT[64:128, qsl], acc_B[HD:2 * HD, :],
                             li[64:128, :])

    # ---- emission ----
    from collections import deque
    phase1_first(xt0)
    _late_consts()
    p1q, prq = deque(), deque()
    for qb in range(NSB):
        if qb + 1 < NSB:
            p1q.extend(make_phase1_tasks(qb + 1))
        accs = attention(qb, p1q, prq)
        normalize(qb, *accs)
        prq.extend(make_proj_tasks(qb, split_evict=qb == NSB - 1))
    while prq:
        prq.popleft()()


def _host_prep(x, Wq, Wk, Wv, Wo):
    xT = np.ascontiguousarray(x.reshape(S, D).T).astype(np.float16)
    jj = np.arange(JBK)[:, None]
    qq = np.arange(JBK)[None, :]
    tri = np.where(jj <= qq, np.float32(0.0), np.float32(NEG))
    nmask = np.concatenate([tri, tri], axis=1)
    in_maps = []
    for c in range(NCORES):
        sl = slice(c * M, (c + 1) * M)
        in_maps.append({
            "xT": xT,
            "wq": np.ascontiguousarray(Wq[sl, :].T).astype(np.float16),
            "wk": np.ascontiguousarray(Wk[sl, :].T).astype(np.float16),
            "wv": np.ascontiguousarray(Wv[sl, :].T).astype(np.float16),
            "wo": np.ascontiguousarray(Wo[:, sl].T).astype(np.float16),
            "ones": np.ones((128, HD), dtype=np.float16),
            "nmask": np.ascontiguousarray(nmask),
        })
    return in_maps


def _run(inputs, trace=False):
    x = np.asarray(inputs["x"], dtype=np.float32)
    Wq = np.asarray(inputs["Wq"], dtype=np.float32)
    Wk = np.asarray(inputs["Wk"], dtype=np.float32)
    Wv = np.asarray(inputs["Wv"], dtype=np.float32)
    Wo = np.asarray(inputs["Wo"], dtype=np.float32)

    if "nc" not in _CACHE:
        _CACHE["nc"] = _build_nc()
    nc = _CACHE["nc"]

    in_maps = _host_prep(x, Wq, Wk, Wv, Wo)
    res = bass_utils.run_bass_kernel_spmd(
        nc, in_maps, core_ids=list(range(NCORES)), trace=trace)
    partial = np.zeros((D, S), dtype=np.float32)
    for c in range(NCORES):
        partial += res.results[c]["outp"].astype(np.float32)
    out = partial.T.astype(np.float32).reshape(B, S, D)
    return out, res


def kernel(x, mask, Wq, Wk, Wv, Wo):
    mask = np.asarray(mask)
    causal = np.tril(np.ones((S, S), dtype=bool))
    if mask.reshape(S, S).shape == causal.shape and bool(
            np.array_equal(mask.reshape(S, S), causal)):
        out, _ = _run({"x": x, "Wq": Wq, "Wk": Wk, "Wv": Wv, "Wo": Wo})
        return out
    # safety net for a non-causal mask: exact numpy fallback
    return _numpy_ref(np.asarray(x, np.float32), mask,
                      np.asarray(Wq, np.float32), np.asarray(Wk, np.float32),
                      np.asarray(Wv, np.float32), np.asarray(Wo, np.float32))


def _numpy_ref(x, mask, Wq, Wk, Wv, Wo):
    xf = x.reshape(S, D)
    q = xf @ Wq.T
    k = xf @ Wk.T
    v = xf @ Wv.T
    m2 = mask.reshape(S, S)
    o = np.empty((S, D), dtype=np.float32)
    for h in range(H):
        hs = slice(h * HD, (h + 1) * HD)
        sc = (q[:, hs] @ k[:, hs].T) / np.sqrt(np.float32(HD))
        sc = np.where(m2, sc, np.float32(-1e9))
        sc -= sc.max(axis=-1, keepdims=True)
        p = np.exp(sc)
        p /= p.sum(axis=-1, keepdims=True)
        o[:, hs] = p @ v[:, hs]
    return (o @ Wo.T).astype(np.float32).reshape(B, S, D)


# revision 18
# speedup vs baseline: 1.3624x; 1.0148x over previous
"""Multi-head causal attention (B=1, S=4096, D=1024, H=16, HD=64) on 8
Trainium2 NeuronCores.

Sharding: head-parallel - 16 heads / 8 cores = 2 heads per core (one
128-channel slice of the QKV/output projections per core).

v2 design notes (vs the f32r baseline at ~310us):
  * All matmul operands are fp16 (psum stays f32). fp16 keeps 1 cyc/row
    streaming, halves LDWEIGHTS (FWL), halves SBUF/DMA bytes, and avoids
    the f32r 4-cyc/row penalty on narrow diagonal tiles. Accuracy budget
    (rel tol 2e-2) has ~20x margin at fp16.
  * Scores use FULL-ARRAY (128-row) stationary kT tiles: trace analysis
    showed full-row LDWEIGHTS pulls ahead under in-flight matmuls
    (phase-1 chains ran at 227ns/MM = stream rate) while the baseline's
    64-row strip LDWs cannot and serialize (~600ns/pair). The two heads
    are separated by ZERO-PADDING the moving Q operand instead: qpad
    holds [qA;0] and [0;qB] blocks so one kT_j stationary serves both
    heads with plain full-array matmuls.
  * Causal masking is additive (-1e5) on the PSUM scores via DVE before
    the exp, removing the GpSimd mask-multiply from the pt->PV critical
    path.
  * Softmax denominator: v_aug = [V_A | ones | V_B] rider (as baseline);
    normalization is DVE reciprocal + multiply (no Ln/Exp table games),
    with two ScalarE Copy ops for the cross-partition l moves.
  * Output projection is interleaved per query-block into the next
    block's attention, evicted on GpSimd, written as fp16 partials.
  * Emission software-pipelines scores(j+1) ahead of PV(j) so the PE
    queue always has independent work while ScalarE runs exp (ScalarE is
    within ~5% of TensorE here; exp is ~110us/core of irreducible work).
"""

import os
import sys

import numpy as np

for _p in ("/opt/trn_rl_repo", "/root/.axon_site/_ro/trn_rl_repo"):
    if os.path.isdir(_p) and _p not in sys.path:
        sys.path.insert(0, _p)

from contextlib import ExitStack

import concourse.bass as bass
import concourse.tile as tile
from concourse import bacc, bass_utils, mybir
from concourse.masks import make_identity

# Problem shape (hardcoded per the harness contract).
B, S, D, H = 1, 4096, 1024, 16
HD = D // H          # 64
NCORES = 8
HPC = H // NCORES    # 2 heads per core
M = HPC * HD         # 128 channels per core
SBK = 512            # query/sequence block size
NSB = S // SBK       # 8
DBK = 128            # d block size
NDB = D // DBK       # 8
JBK = 128            # key block size
NJT = S // JBK       # 32 j-tiles
VW = 3 * HD          # v_aug row width per j-tile: [V_A | ones | V_B]
NEG = -1.0e5         # additive causal mask value (pre-softmax)

F32 = mybir.dt.float32
F16 = mybir.dt.float16
F32R = mybir.dt.float32r

_CACHE = {}


def _build_nc():
    nc = bacc.Bacc("TRN2", target_bir_lowering=False, debug=False,
                   num_devices=NCORES)

    xT = nc.dram_tensor("xT", [D, S], F16, kind="ExternalInput").ap()
    wq = nc.dram_tensor("wq", [D, M], F16, kind="ExternalInput").ap()
    wk = nc.dram_tensor("wk", [D, M], F16, kind="ExternalInput").ap()
    wv = nc.dram_tensor("wv", [D, M], F16, kind="ExternalInput").ap()
    wo = nc.dram_tensor("wo", [M, D], F16, kind="ExternalInput").ap()
    nmask = nc.dram_tensor("nmask", [JBK, 2 * JBK], F32,
                           kind="ExternalInput").ap()
    outp = nc.dram_tensor("outp", [D, S], F16, kind="ExternalOutput").ap()

    with tile.TileContext(nc) as tc:
        with ExitStack() as ctx:
            _emit(ctx, tc, nc, xT, wq, wk, wv, wo, nmask, outp)
    nc.compile()
    return nc


def _emit(ctx, tc, nc, xT, wq, wk, wv, wo, nmask, outp):
    const = ctx.enter_context(tc.tile_pool(name="const", bufs=1))
    persist = ctx.enter_context(tc.tile_pool(name="persist", bufs=1))
    xt_pool = ctx.enter_context(tc.tile_pool(name="xt", bufs=3))
    vt_pool = ctx.enter_context(tc.tile_pool(name="vt", bufs=2))
    pt_pool = ctx.enter_context(tc.tile_pool(name="pt", bufs=4))
    out_pool = ctx.enter_context(tc.tile_pool(name="outt", bufs=4))
    small = ctx.enter_context(tc.tile_pool(name="small", bufs=2))
    # PSUM budget (16KB/partition = 8 banks):
    #   psA tag "sc":  [128,1024] f32 = 4KB x2 bufs = 8KB (scores; phase1 q/k)
    #   psB tag "acc": [128,512]  f32 = 2KB x2 bufs = 4KB (acc A/B; phase1 v)
    #   psB tag "po":  [128,512]  f32 = 2KB x2 bufs = 4KB (proj out)
    psA = ctx.enter_context(tc.tile_pool(name="psA", bufs=2, space="PSUM"))
    psB = ctx.enter_context(tc.tile_pool(name="psB", bufs=2, space="PSUM"))

    # ---- constants / persistent SBUF ----
    ident = const.tile([128, 128], F16)

    wq_sb = const.tile([128, D], F16)    # 8 d-tiles side by side [d, m]
    wk_sb = const.tile([128, D], F16)
    wv_sb = const.tile([128, D], F16)
    wo_sb = const.tile([128, D], F16)    # [m, o]
    mask_sb = const.tile([JBK, 2 * JBK], F32)

    for w_sb, w_dram in ((wq_sb, wq), (wk_sb, wk), (wv_sb, wv)):
        w_r = w_dram.rearrange("(d p) m -> p d m", p=DBK)
        w_sb_r = w_sb[:].rearrange("p (d m) -> p d m", d=NDB)
        nc.sync.dma_start(out=w_sb_r[:, 0:4, :], in_=w_r[:, 0:4, :])
        nc.sync.dma_start(out=w_sb_r[:, 4:8, :], in_=w_r[:, 4:8, :])

    def _late_consts():
        nc.sync.dma_start(out=mask_sb[:], in_=nmask[:])
        nc.sync.dma_start(out=wo_sb[:], in_=wo[:])

    # qpad: per query-block layout [qA(512) ; zeros] | [zeros ; qB(512)]
    qpad = persist.tile([128, 2 * S], F16)
    kT_sb = persist.tile([128, S], F16)
    v_aug = persist.tile([128, NJT * VW], F16)
    attnT = persist.tile([128, S], F16)

    # the first xt DMA must be the first GpSimd op so phase 1 starts
    # immediately; ident/memset follow it in the GpSimd queue.
    xt0 = xt_pool.tile([128, NDB * SBK], F16, tag="xt")
    xT_r0 = xT.rearrange("(d p) s -> p d s", p=DBK)
    xt0_r = xt0[:].rearrange("p (d s) -> p d s", d=NDB)
    nc.gpsimd.dma_start(out=xt0_r[:, 0:4, :], in_=xT_r0[:, 0:4, 0:SBK])
    nc.gpsimd.dma_start(out=xt0_r[:, 4:8, :], in_=xT_r0[:, 4:8, 0:SBK])
    make_identity(nc, ident)
    nc.vector.memset(qpad[:], 0.0)

    # v_aug per j-tile: [V_A | ones | V_B]; head A reads cols 0:128
    # (denominator in acc rows 64:128), head B reads cols 64:192
    # (denominator in acc rows 0:64). ones lane filled by one memset.
    v_aug_r = v_aug[:].rearrange("p (t c w) -> p t c w", c=3, w=HD)
    nc.gpsimd.memset(v_aug_r[:, :, 1, :], 1.0)

    def phase1_first(xt):
        """QKV projections for s-block 0, run before attention starts."""
        q_ps = psA.tile([128, SBK], F32, tag="sc")
        k_ps = psA.tile([128, SBK], F32, tag="sc")
        v_ps = psB.tile([128, SBK], F32, tag="acc")
        for d in range(NDB):
            st, sp = d == 0, d == NDB - 1
            nc.tensor.matmul(q_ps[:], lhsT=wq_sb[:, bass.ts(d, M)],
                             rhs=xt[:, bass.ts(d, SBK)], start=st, stop=sp)
            nc.tensor.matmul(k_ps[:], lhsT=wk_sb[:, bass.ts(d, M)],
                             rhs=xt[:, bass.ts(d, SBK)], start=st, stop=sp)
            nc.tensor.matmul(v_ps[:], lhsT=wv_sb[:, bass.ts(d, M)],
                             rhs=xt[:, bass.ts(d, SBK)], start=st, stop=sp)
        nc.scalar.copy(qpad[0:64, 0:SBK], q_ps[0:64, :])
        nc.scalar.copy(qpad[64:128, SBK:2 * SBK], q_ps[64:128, :])
        nc.scalar.copy(kT_sb[:, 0:SBK], k_ps[:])
        vt = vt_pool.tile([128, SBK], F16, tag="vt")
        nc.vector.tensor_copy(vt[:], v_ps[:])
        for t in range(SBK // JBK):
            tp = psB.tile([128, JBK], F16, tag="po")
            nc.tensor.transpose(tp[:], vt[:, bass.ts(t, JBK)], ident[:])
            nc.vector.tensor_copy(v_aug_r[:, t, 0::2, :], tp[:])

    def make_phase1_tasks(sb):
        """Phase-1 work for s-block sb as PE side-tasks (run interleaved
        into the previous attention block; evictions on DVE)."""
        xt = xt_pool.tile([128, NDB * SBK], F16, tag="xt", name=f"xt{sb}")
        xT_r = xT.rearrange("(d p) s -> p d s", p=DBK)
        nc.gpsimd.dma_start(out=xt[:].rearrange("p (d s) -> p d s", d=NDB),
                            in_=xT_r[:, :, sb * SBK:(sb + 1) * SBK])
        st_ = {}

        def chain_half(kind, w_sb, lo):
            def t():
                if lo == 0:
                    st_[kind] = psB.tile([128, SBK], F32, tag="po",
                                         name=f"p1{kind}{sb}")
                ps = st_[kind]
                for d in range(lo, lo + 4):
                    nc.tensor.matmul(ps[:], lhsT=w_sb[:, bass.ts(d, M)],
                                     rhs=xt[:, bass.ts(d, SBK)],
                                     start=d == 0, stop=d == NDB - 1)
                if lo == 4:
                    if kind == "q":
                        nc.vector.tensor_copy(
                            qpad[0:64, 2 * sb * SBK:(2 * sb + 1) * SBK],
                            ps[0:64, :])
                        nc.vector.tensor_copy(
                            qpad[64:128, (2 * sb + 1) * SBK:(2 * sb + 2) * SBK],
                            ps[64:128, :])
                    elif kind == "k":
                        nc.vector.tensor_copy(kT_sb[:, bass.ts(sb, SBK)],
                                              ps[:])
                    else:
                        vt = vt_pool.tile([128, SBK], F16, tag="vt", name=f"vt{sb}")
                        nc.vector.tensor_copy(vt[:], ps[:])
                        st_["vt"] = vt
            return t

        def tp_pair(t0):
            def t():
                vt = st_["vt"]
                for tt in (t0, t0 + 1):
                    tp = psB.tile([128, JBK], F16, tag="po",
                                  name=f"tp{sb}_{tt}")
                    nc.tensor.transpose(tp[:], vt[:, bass.ts(tt, JBK)],
                                        ident[:])
                    nc.vector.tensor_copy(
                        v_aug_r[:, sb * (SBK // JBK) + tt, 0::2, :], tp[:])
            return t

        return ([(sb, k, chain_half(k, w, lo))
                 for k, w in (("q", wq_sb), ("k", wk_sb), ("v", wv_sb))
                 for lo in (0, 4)]
                + [(sb, "tp", tp_pair(0)), (sb, "tp", tp_pair(2))])

    def make_proj_tasks(qb, split_evict=False):
        """Output-projection partial for query block qb (one task per
        128-wide output slice; eviction on DVE, store on Sync). With
        split_evict, alternate evictions between ScalarE and DVE (used
        for the final block where both engines are otherwise idle)."""
        qsl = bass.ts(qb, SBK)

        def mk(ob):
            def t():
                if split_evict and ob % 2 == 0:
                    po = psA.tile([128, SBK], F32, tag="sc",
                                  name=f"po{qb}_{ob}")
                else:
                    po = psB.tile([128, SBK], F32, tag="po",
                                  name=f"po{qb}_{ob}")
                nc.tensor.matmul(po[:], lhsT=wo_sb[:, bass.ts(ob, 128)],
                                 rhs=attnT[:, qsl], start=True, stop=True)
                ot = out_pool.tile([128, SBK], F16, tag="ot", name=f"ot{qb}_{ob}")
                if split_evict and ob % 2 == 0:
                    nc.scalar.copy(ot[:], po[:])
                else:
                    nc.vector.tensor_copy(ot[:], po[:])
                nc.sync.dma_start(out=outp[bass.ts(ob, 128), qsl], in_=ot[:])
            return t

        return [mk(ob) for ob in range(NDB)]

    def attention(qb, p1q, prq):
        """Causal attention for query block qb (both heads). Pops side
        tasks (phase-1 chains, projections) between the score and PV
        matmuls of each j-iteration so they fill the exp-wait bubble."""
        nj = 4 * (qb + 1)
        acc_A = psB.tile([128, SBK], F32, tag="acc")
        acc_B = psB.tile([128, SBK], F32, tag="acc")

        def emit_scores(j):
            r = j - (nj - 4)
            off = 128 * r if r > 0 else 0
            sc = psA.tile([128, 2 * SBK], F32, tag="sc")
            kslice = kT_sb[:, bass.ts(j, JBK)]
            qa = qpad[:, 2 * qb * SBK + off:(2 * qb + 1) * SBK]
            qb_ap = qpad[:, (2 * qb + 1) * SBK + off:(2 * qb + 2) * SBK]
            nc.tensor.matmul(sc[:, off:SBK], lhsT=kslice, rhs=qa,
                             start=True, stop=True)
            nc.tensor.matmul(sc[:, SBK + off:2 * SBK], lhsT=kslice,
                             rhs=qb_ap, start=True, stop=True)
            if r >= 0:
                # additive causal mask on the [128,128] diagonal strip of
                # both heads (one 3D DVE op, pre-exp, on PSUM).
                dlo = 128 * r
                sc3 = bass.AP(tensor=sc.tensor, offset=sc.offset + dlo,
                              ap=[list(sc.ap[0]), [SBK, 2], [1, JBK]])
                m3 = mask_sb[:].rearrange("p (b c) -> p b c", b=2)
                nc.vector.tensor_add(sc3, sc3, m3)
            return sc, off

        cur = emit_scores(0)
        for j in range(nj):
            if j + 1 < nj:
                if j + 1 >= nj - 4:
                    # the diagonal j-tiles read kT/v_aug of s-block qb:
                    # force-complete all phase-1 work for sb <= qb first.
                    while p1q and p1q[0][0] <= qb:
                        p1q.popleft()[2]()
                nxt = emit_scores(j + 1)
            else:
                nxt = None
            if p1q:
                p1q.popleft()[2]()
            elif prq:
                prq.popleft()()
                if prq:
                    prq.popleft()()
            sc, off = cur
            pt = pt_pool.tile([128, 2 * SBK], F16, tag="pt")
            scale = float(1.0 / np.sqrt(HD))
            if off == 0:
                nc.scalar.activation(pt[:], sc[:],
                                     mybir.ActivationFunctionType.Exp,
                                     scale=scale)
            else:
                w = SBK - off
                sc2 = bass.AP(tensor=sc.tensor, offset=sc.offset + off,
                              ap=[list(sc.ap[0]), [SBK, 2], [1, w]])
                pt2 = bass.AP(tensor=pt.tensor, offset=pt.offset + off,
                              ap=[list(pt.ap[0]), [SBK, 2], [1, w]])
                nc.scalar.activation(pt2, sc2,
                                     mybir.ActivationFunctionType.Exp,
                                     scale=scale)
            st, sp = j == 0, j == nj - 1
            vb = j * VW
            nc.tensor.matmul(acc_A[:, off:SBK],
                             lhsT=v_aug[:, vb:vb + 128],
                             rhs=pt[:, off:SBK], start=st, stop=sp)
            nc.tensor.matmul(acc_B[:, off:SBK],
                             lhsT=v_aug[:, vb + HD:vb + VW],
                             rhs=pt[:, SBK + off:2 * SBK],
                             start=st, stop=sp)
            cur = nxt
        # the next block's scores need its qpad half: finish the q-chain
        while p1q and p1q[0][0] == qb + 1 and p1q[0][1] == "q":
            p1q.popleft()[2]()
        return acc_A, acc_B

    def normalize(qb, acc_A, acc_B):
        """attnT = acc_out / l. Head A: out rows 0:64, l rows 64:128;
        head B flipped. Cross-partition l moves on ScalarE Copy, then one
        DVE reciprocal + two DVE multiplies."""
        qsl = bass.ts(qb, SBK)
        lt = small.tile([128, SBK], F32, tag="lt")
        nc.scalar.copy(lt[0:64, :], acc_A[HD:2 * HD, :])
        nc.scalar.copy(lt[64:128, :], acc_B[0:HD, :])
        li = small.tile([128, SBK], F32, tag="li")
        nc.vector.reciprocal_approx_fast(out=li[:], in_=lt[:])
        nc.vector.tensor_mul(attnT[0:64, qsl], acc_A[0:HD, :], li[0:64, :])
        nc.vector.tensor_mul(attnT[64:128, qsl], acc_B[HD:2 * HD, :],
                             li[64:128, :])

    # ---- emission ----
    from collections import deque
    phase1_first(xt0)
    _late_consts()
    p1q, prq = deque(), deque()
    for qb in range(NSB):
        if qb + 1 < NSB:
            p1q.extend(make_phase1_tasks(qb + 1))
        accs = attention(qb, p1q, prq)
        normalize(qb, *accs)
        prq.extend(make_proj_tasks(qb, split_evict=qb == NSB - 1))
    while prq:
        prq.popleft()()


def _host_prep(x, Wq, Wk, Wv, Wo):
    xT = np.ascontiguousarray(x.reshape(S, D).T).astype(np.float16)
    jj = np.arange(JBK)[:, None]
    qq = np.arange(JBK)[None, :]
    tri = np.where(jj <= qq, np.float32(0.0), np.float32(NEG))
    nmask = np.concatenate([tri, tri], axis=1)
    in_maps = []
    for c in range(NCORES):
        sl = slice(c * M, (c + 1) * M)
        in_maps.append({
            "xT": xT,
            "wq": np.ascontiguousarray(Wq[sl, :].T).astype(np.float16),
            "wk": np.ascontiguousarray(Wk[sl, :].T).astype(np.float16),
            "wv": np.ascontiguousarray(Wv[sl, :].T).astype(np.float16),
            "wo": np.ascontiguousarray(Wo[:, sl].T).astype(np.float16),
            "nmask": np.ascontiguousarray(nmask),
        })
    return in_maps


def _run(inputs, trace=False):
    x = np.asarray(inputs["x"], dtype=np.float32)
    Wq = np.asarray(inputs["Wq"], dtype=np.float32)
    Wk = np.asarray(inputs["Wk"], dtype=np.float32)
    Wv = np.asarray(inputs["Wv"], dtype=np.float32)
    Wo = np.asarray(inputs["Wo"], dtype=np.float32)

    if "nc" not in _CACHE:
        _CACHE["nc"] = _build_nc()
    nc = _CACHE["nc"]

    in_maps = _host_prep(x, Wq, Wk, Wv, Wo)
    res = bass_utils.run_bass_kernel_spmd(
        nc, in_maps, core_ids=list(range(NCORES)), trace=trace)
    partial = np.zeros((D, S), dtype=np.float32)
    for c in range(NCORES):
        partial += res.results[c]["outp"].astype(np.float32)
    out = partial.T.astype(np.float32).reshape(B, S, D)
    return out, res


def kernel(x, mask, Wq, Wk, Wv, Wo):
    mask = np.asarray(mask)
    causal = np.tril(np.ones((S, S), dtype=bool))
    if mask.reshape(S, S).shape == causal.shape and bool(
            np.array_equal(mask.reshape(S, S), causal)):
        out, _ = _run({"x": x, "Wq": Wq, "Wk": Wk, "Wv": Wv, "Wo": Wo})
        return out
    # safety net for a non-causal mask: exact numpy fallback
    return _numpy_ref(np.asarray(x, np.float32), mask,
                      np.asarray(Wq, np.float32), np.asarray(Wk, np.float32),
                      np.asarray(Wv, np.float32), np.asarray(Wo, np.float32))


def _numpy_ref(x, mask, Wq, Wk, Wv, Wo):
    xf = x.reshape(S, D)
    q = xf @ Wq.T
    k = xf @ Wk.T
    v = xf @ Wv.T
    m2 = mask.reshape(S, S)
    o = np.empty((S, D), dtype=np.float32)
    for h in range(H):
        hs = slice(h * HD, (h + 1) * HD)
        sc = (q[:, hs] @ k[:, hs].T) / np.sqrt(np.float32(HD))
        sc = np.where(m2, sc, np.float32(-1e9))
        sc -= sc.max(axis=-1, keepdims=True)
        p = np.exp(sc)
        p /= p.sum(axis=-1, keepdims=True)
        o[:, hs] = p @ v[:, hs]
    return (o @ Wo.T).astype(np.float32).reshape(B, S, D)
